# revision 1
# baseline (speedup 1.0000x reference)
"""GQA kernel for trn2, 8 NeuronCores — bf16, phase-interleaved.

Sharding: core c = (b, g2) with b = c//4, g2 = c%4.  Each core handles batch b
and kv heads {2*g2, 2*g2+1} (q heads 8*g2 .. 8*g2+7).  Wq/Wk/Wv column-sharded
(packed as one wqkv [D,768] tensor), Wo row-sharded; host sums the 4 partial
y outputs per batch (y is emitted bf16, upcast on host).

Everything the PE touches is bf16 (full rate at any moving width, half the
DMA bytes); psum accumulation is f32.  Program order interleaves phases
  A0 A1 B0 A2 B1+C A3 B2+C B3+C Ctail
so the PE never drains: A(sb) = qkv projection + RoPE for one 512-row
s-block, B(bi) = attention for one 512-row i-band (ACT exp paced), C chunks
= output-projection tiles threaded into B's exp-latency slots.

The two per-head exps per j-tile are fused into one joint ACT call on an
adjacent psum pair.  Softmax normalization per (bi,t): the pv matmul's extra
ones-row gives the denominator, DVE reciprocal, then an outer-product matmul
(ones column x recip row) broadcasts it across partitions — no DRAM round
trip.  V is PE-transposed into a bitcast bf16 psum slice.  RoPE rotate-half
is 4 partition-block-copy DMAs per tensor.
"""

import os
import numpy as np
import ml_dtypes

import concourse.bass as bass
import concourse.bacc as bacc
import concourse.mybir as mybir
import concourse.tile as tile
from concourse.bass_utils import run_bass_kernel_spmd

F32 = mybir.dt.float32
BF16 = mybir.dt.bfloat16

B, S, D = 2, 2048, 2048
H, KV, HD = 32, 8, 64
N_CORES = 8
SB = 512
NSB = S // SB     # 4
NDT = D // 128    # 16
NET = 4           # q e-tiles per core
NIT = S // SB     # 4
NJT = S // 128    # 16
PERM = [0, 4, 1, 5, 2, 6, 3, 7]
EXP = mybir.ActivationFunctionType.Exp

LAST_RESULT = None


def build_nc():
    nc = bacc.Bacc("TRN2", target_bir_lowering=False, debug=False,
                   enable_asserts=True, num_devices=N_CORES)

    xT = nc.dram_tensor("xT", [D, S], BF16, kind="ExternalInput")
    wqkv = nc.dram_tensor("wqkv", [D, 768], BF16, kind="ExternalInput")
    wo = nc.dram_tensor("wo", [512, D], BF16, kind="ExternalInput")
    cos2 = nc.dram_tensor("cos2", [128, S], BF16, kind="ExternalInput")
    sin2 = nc.dram_tensor("sin2", [128, S], BF16, kind="ExternalInput")
    cmask = nc.dram_tensor("cmask", [128, SB], BF16, kind="ExternalInput")
    ident = nc.dram_tensor("ident", [128, 128], BF16, kind="ExternalInput")
    y = nc.dram_tensor("y", [S, D], BF16, kind="ExternalOutput")
    rscratch = nc.dram_tensor("rscratch", [NIT, NET, 2, SB], BF16)  # internal

    with tile.TileContext(nc) as tc:
        with (
            tc.tile_pool(name="persist", bufs=1) as persist,
            tc.tile_pool(name="wpool", bufs=1) as wpool,
            tc.tile_pool(name="xpool", bufs=1) as xpool,
            tc.tile_pool(name="apool", bufs=2, space="PSUM") as apool,
            tc.tile_pool(name="scpool", bufs=2, space="PSUM") as scpool,
            tc.tile_pool(name="pvpool", bufs=1, space="PSUM") as pvpool,
            tc.tile_pool(name="atmp", bufs=3) as atmp,
            tc.tile_pool(name="epool", bufs=6) as epool,
            tc.tile_pool(name="btmp", bufs=2) as btmp,
            tc.tile_pool(name="ypool", bufs=2) as ypool,
        ):
            # ---- persistent SBUF ----
            # per-s-block tiles: avoids false inter-phase deps from
            # tile-granular dependency tracking
            qT_sb = [[persist.tile([128, SB], BF16, name=f"qT{t}_{s_}")
                      for s_ in range(NSB)] for t in range(NET)]
            kT_sb = [persist.tile([128, SB], BF16, name=f"kT{s_}")
                     for s_ in range(NSB)]
            v_ones0 = [persist.tile([128, 4, 65], BF16, name=f"v_ones0_{s_}")
                       for s_ in range(NSB)]
            v_ones1 = [persist.tile([128, 4, 65], BF16, name=f"v_ones1_{s_}")
                       for s_ in range(NSB)]
            outT = [[persist.tile([128, SB], BF16, name=f"outT{t}_{s_}")
                     for s_ in range(NSB)] for t in range(NET)]
            cos_sb = persist.tile([128, S], BF16, name="cos_sb")
            sin_sb = persist.tile([128, S], BF16, name="sin_sb")
            cmask_sb = persist.tile([128, SB], BF16, name="cmask_sb")
            ident_sb = persist.tile([128, 128], BF16, name="ident_sb")
            ones_col = persist.tile([128, 4, 1], BF16, name="ones_col")
            ones_sb = persist.tile([128, 64], BF16, name="ones_sb")
            nc.gpsimd.memset(ones_col[:], 1.0)
            nc.gpsimd.memset(ones_sb[:], 1.0)
            for s_ in range(NSB):
                nc.vector.tensor_copy(v_ones0[s_][:, :, 64:65], ones_col[:])
                nc.vector.tensor_copy(v_ones1[s_][:, :, 64:65], ones_col[:])

            wqkv_sb = [wpool.tile([128, 768], BF16, name=f"wqkv{d}") for d in range(NDT)]
            wo_sb = [wpool.tile([128, D], BF16, name=f"wo{f}") for f in range(4)]
            xs = [xpool.tile([128, 2, 1024], BF16, name=f"x{d}") for d in range(NDT)]

            # ---- prefetch DMAs ----
            # weights on the ACT queue (ACT idle until B0), x/consts on SP.
            for d in range(NDT):
                nc.scalar.dma_start(wqkv_sb[d][:], wqkv[d * 128:(d + 1) * 128, :])
                nc.sync.dma_start(xs[d][:, 0, :], xT[d * 128:(d + 1) * 128, 0:1024])
                if d == 7:
                    nc.scalar.dma_start(cos_sb[:], cos2[:])
                    nc.scalar.dma_start(sin_sb[:], sin2[:])
                if d == 11:
                    nc.scalar.dma_start(ident_sb[:], ident[:])
            nc.scalar.dma_start(cmask_sb[:], cmask[:])

            tri = cmask_sb[:, 0:128]

            # ---------------- phase A ----------------
            # generator: yields after every couple of matmuls so phase B can
            # thread these steps into its exp-latency slots
            def gen_A(sb):
                scol = slice(sb * SB, (sb + 1) * SB)
                half, xoff = sb // 2, (sb % 2) * 512
                xsl = lambda d: xs[d][:, half, xoff:xoff + 512]
                if sb == 1:
                    # x cols 1024:2048 are needed only from A2 on; wo only at
                    # C0.  Issuing them early clogs the DMA engines and blocks
                    # A0's RoPE shift DMAs, which gate B0's start.
                    for d in range(NDT):
                        nc.sync.dma_start(xs[d][:, 1, :],
                                          xT[d * 128:(d + 1) * 128, 1024:2048])
                # chains: k first (B needs it earliest), then q0..q3, then v
                chains = [("k", slice(512, 640))] + \
                         [(t, slice(t * 128, (t + 1) * 128)) for t in range(NET)] + \
                         [("v", slice(640, 768))]
                if sb == 0:
                    # d-major for the first s-block: x tiles arrive slower than
                    # one chain consumes them, so run all 6 chains per tile.
                    # B-phase psum slots are idle here — borrow them.
                    scjt = scpool.tile([128, 2, SB], F32, name="scja", tag="scj")
                    accs = [apool.tile([128, SB], F32, name="acc", tag="acc"),
                            apool.tile([128, SB], F32, name="acc", tag="acc"),
                            pvpool.tile([128, SB], F32, name="pva", tag="pvA"),
                            pvpool.tile([128, SB], F32, name="pvb", tag="pvB"),
                            scjt[:, 0, :], scjt[:, 1, :]]
                    accof = {o: accs[ci] for ci, (o, wcol) in enumerate(chains)}
                    for d in range(NDT):
                        st, sp = d == 0, d == NDT - 1
                        for ci, (o, wcol) in enumerate(chains):
                            ac = accof[o]
                            av = ac[:] if hasattr(ac, "tensor") else ac
                            nc.tensor.matmul(av, wqkv_sb[d][:, wcol], xsl(d),
                                             start=st, stop=sp)
                    accs_by_chain = accof
                else:
                    accs_by_chain = None
                for ci, (o, wcol) in enumerate(chains):
                    if accs_by_chain is None:
                        acc = apool.tile([128, SB], F32, name="acc", tag="acc")
                        for d0 in range(0, NDT, 2):
                            for d in (d0, d0 + 1):
                                nc.tensor.matmul(acc[:], wqkv_sb[d][:, wcol],
                                                 xsl(d), start=(d == 0),
                                                 stop=(d == NDT - 1))
                            yield
                    else:
                        ac = accs_by_chain[o]
                        acc = ac if not hasattr(ac, "tensor") else ac
                    if o == "v":
                        vtmp = atmp.tile([128, SB], BF16, name="vtmp", tag="vtmp",
                                         bufs=2)
                        if sb == 0:
                            nc.scalar.copy(vtmp[:], acc[:])
                        else:
                            nc.vector.tensor_copy(vtmp[:], acc[:])
                        if sb == 0:
                            # borrow B-phase slots so A1's chains don't queue
                            # behind the transpose drains on the acc ring
                            scjt2 = scpool.tile([128, 2, SB], F32, name="scjb",
                                                tag="scj")
                            trts = [pvpool.tile([128, SB], F32, name="pvat",
                                                tag="pvA"),
                                    pvpool.tile([128, SB], F32, name="pvbt",
                                                tag="pvB"),
                                    scjt2[:, 0, :], scjt2[:, 1, :]]
                        else:
                            trts = None
                        for u in range(4):
                            usl = slice(u * 128, (u + 1) * 128)
                            if trts is None:
                                trt = apool.tile([128, SB], F32, name="tr",
                                                 tag="acc")[:]
                            else:
                                ak = trts[u]
                                trt = ak[:] if hasattr(ak, "tensor") else ak
                            tr = trt[:, 0:64].bitcast(BF16)
                            nc.tensor.transpose(tr[:], vtmp[:, usl], ident_sb[:])
                            nc.vector.tensor_copy(v_ones0[sb][:, u, 0:64], tr[:, 0:64])
                            nc.vector.tensor_copy(v_ones1[sb][:, u, 0:64], tr[:, 64:128])
                    else:
                        dst = kT_sb[sb] if o == "k" else qT_sb[o][sb]
                        qtmp = atmp.tile([128, SB], BF16, name="qtmp", tag="qtmp",
                                         bufs=4)
                        if sb == 0:
                            nc.scalar.copy(qtmp[:], acc[:])
                        else:
                            nc.vector.tensor_copy(qtmp[:], acc[:])
                        rot = atmp.tile([128, SB], BF16, name="rot", tag="rot")
                        # rotate-half: swap 32-partition blocks (0<->32, 64<->96)
                        for (a, b_) in ((0, 32), (32, 0), (64, 96), (96, 64)):
                            nc.sync.dma_start(rot[b_:b_ + 32, :], qtmp[a:a + 32, :])
                        t1 = atmp.tile([128, SB], BF16, name="t1", tag="t1", bufs=2)
                        nc.vector.tensor_mul(t1[:], qtmp[:], cos_sb[:, scol])
                        t2 = atmp.tile([128, SB], BF16, name="t2", tag="t2", bufs=2)
                        nc.vector.tensor_mul(t2[:], rot[:], sin_sb[:, scol])
                        nc.vector.tensor_add(dst[:], t1[:], t2[:])
                    yield

            # ---------------- phase C chunks ----------------
            c_state = {"ys": None}

            def emit_C_chunk(stt, db):
                srow = slice(stt * 128, (stt + 1) * 128)
                dcol = slice(db * SB, (db + 1) * SB)
                yp = apool.tile([128, SB], F32, name="yp", tag="acc")
                ssl = slice((stt % 4) * 128, (stt % 4 + 1) * 128)
                for f in range(4):
                    nc.tensor.matmul(yp[:], outT[f][stt // 4][:, ssl],
                                     wo_sb[f][:, dcol],
                                     start=(f == 0), stop=(f == 3))
                if db == 0:
                    c_state["ys"] = ypool.tile([128, D], BF16, name="ys", tag="ys")
                ys = c_state["ys"]
                if c_state.get("tail"):
                    nc.scalar.copy(ys[:, dcol], yp[:])
                else:
                    nc.vector.tensor_copy(ys[:, dcol], yp[:])
                nc.sync.dma_start(y[srow, db * SB:(db + 1) * SB], ys[:, dcol])

            c_chunks = []          # ready-to-emit (stt, db) list, FIFO

            def c_hook():
                if c_chunks:
                    emit_C_chunk(*c_chunks.pop(0))

            # ---------------- phase B ----------------
            def emit_B(bi, fillers=(), every=4):
                if bi == 0:
                    for f_ in range(4):
                        nc.sync.dma_start(wo_sb[f_][:],
                                          wo[f_ * 128:(f_ + 1) * 128, :])
                icol = slice(bi * SB, (bi + 1) * SB)
                njt = 4 * bi + 4
                for t in range(NET):
                    if t > 0:
                        # cover the previous pv pair's drain latency
                        # (pvpool bufs=1) with independent filler work
                        for f in fillers:
                            if f() and f():
                                break
                    pvA = pvpool.tile([65, SB], F32, name="pvA", tag="pvA")
                    pvB = pvpool.tile([65, SB], F32, name="pvB", tag="pvB")
                    for jt in range(njt):
                        js, ju = jt // 4, jt % 4
                        jcol = slice(ju * 128, (ju + 1) * 128)
                        ro = jt - 4 * bi
                        lo = 128 * max(ro, 0)
                        kt, qt = kT_sb[js], qT_sb[t][bi]
                        scj = scpool.tile([128, 2, SB], F32, name="scj", tag="scj")
                        nc.tensor.matmul(scj[:, 0, lo:], kt[0:64, jcol],
                                         qt[0:64, lo:], start=True, stop=True)
                        nc.tensor.matmul(scj[:, 1, lo:], kt[64:128, jcol],
                                         qt[64:128, lo:], start=True, stop=True)
                        ej = epool.tile([128, 2, SB], BF16, name="ej", tag="ej")
                        nc.scalar.activation(ej[:, :, lo:], scj[:, :, lo:],
                                             EXP, scale=0.125)
                        eA, eB = ej[:, 0, :], ej[:, 1, :]
                        if ro >= 0:
                            nc.vector.tensor_mul(eA[:, lo:lo + 128],
                                                 eA[:, lo:lo + 128], tri)
                            nc.vector.tensor_mul(eB[:, lo:lo + 128],
                                                 eB[:, lo:lo + 128], tri)
                        st, sp = jt == 0, jt == njt - 1
                        nc.tensor.matmul(pvA[:, lo:], v_ones0[js][:, ju, :],
                                         eA[:, lo:], start=st, stop=sp)
                        nc.tensor.matmul(pvB[:, lo:], v_ones1[js][:, ju, :],
                                         eB[:, lo:], start=st, stop=sp)
                        if jt % every == every - 1:
                            for f in fillers:
                                if f():
                                    break
                    # drains: outT halves, softmax denominators, normalization
                    nc.vector.tensor_copy(outT[t][bi][0:64, :], pvA[0:64, :])
                    pvsB = btmp.tile([65, SB], BF16, name="pvsB", tag="pvsB")
                    nc.vector.tensor_copy(pvsB[:], pvB[:])
                    nc.sync.dma_start(outT[t][bi][64:128, :], pvsB[0:64, :])
                    rAB = btmp.tile([128, 2, SB], BF16, name="rAB", tag="rAB")
                    with nc.allow_low_precision(reason="bf16 softmax recip"):
                        nc.vector.reciprocal(rAB[64:65, 0, :], pvA[64:65, :])
                        nc.vector.reciprocal(rAB[64:65, 1, :], pvsB[64:65, :])
                    if bi < 3:
                        # partition-broadcast via DRAM round-trip on the idle
                        # Pool queue (frees the PE outer-product matmuls);
                        # bf16 bc also gives the norm muls DVE 2x mode
                        nc.gpsimd.dma_start(rscratch[bi, t], rAB[64:65, :, :])
                        bc = btmp.tile([128, SB], BF16, name="bc", tag="bc",
                                       bufs=3)
                        nc.gpsimd.dma_start(
                            bc[0:64, :],
                            rscratch[bi, t, 0:1, :].broadcast_to((64, SB)))
                        nc.gpsimd.dma_start(
                            bc[64:128, :],
                            rscratch[bi, t, 1:2, :].broadcast_to((64, SB)))
                        nc.vector.tensor_mul(outT[t][bi][0:64, :],
                                             outT[t][bi][0:64, :], bc[0:64, :])
                        nc.vector.tensor_mul(outT[t][bi][64:128, :],
                                             outT[t][bi][64:128, :],
                                             bc[64:128, :])
                    else:
                        # bi=3 feeds the tail C chunks: use the low-latency
                        # PE outer-product broadcast instead of the DMA
                        # round-trip so the tail doesn't stall
                        bcp = apool.tile([128, SB], F32, name="bcp", tag="acc")
                        nc.tensor.matmul(bcp[0:64, :], ones_sb[64:65, :],
                                         rAB[64:65, 0, :], start=True, stop=True)
                        nc.tensor.matmul(bcp[64:128, :], ones_sb[64:65, :],
                                         rAB[64:65, 1, :], start=True, stop=True)
                        nc.vector.tensor_mul(outT[t][bi][0:64, :],
                                             outT[t][bi][0:64, :], bcp[0:64, :])
                        nc.vector.tensor_mul(outT[t][bi][64:128, :],
                                             outT[t][bi][64:128, :],
                                             bcp[64:128, :])
                # this bi's output rows are ready for phase C
                for stt in range(4 * bi, 4 * bi + 4):
                    for db in range(4):
                        c_chunks.append((stt, db))

            # ---------------- program order ----------------
            def run_A(sb):
                for _ in gen_A(sb):
                    pass

            gA = {"g": None}

            def a_filler():
                if gA["g"] is None:
                    return False
                try:
                    next(gA["g"])
                    return True
                except StopIteration:
                    gA["g"] = None
                    return False

            def c_filler():
                if c_chunks:
                    emit_C_chunk(*c_chunks.pop(0))
                    return True
                return False

            run_A(0)
            run_A(1)
            gA["g"] = gen_A(2)
            emit_B(0, fillers=[lambda: bool(a_filler()) | bool(a_filler())],
                   every=1)
            while a_filler():
                pass
            gA["g"] = gen_A(3)
            emit_B(1, fillers=[a_filler, c_filler], every=1)
            while a_filler():
                pass
            emit_B(2, fillers=[c_filler], every=4)
            emit_B(3, fillers=[c_filler], every=3)
            c_state["tail"] = True
            while c_chunks:
                emit_C_chunk(*c_chunks.pop(0))

    nc.compile()
    return nc


def host_inputs(x, Wq, Wk, Wv, Wo):
    """Per-core input maps (8 cores)."""
    BF = ml_dtypes.bfloat16
    inv = 1.0 / (10000.0 ** (np.arange(0, HD, 2, dtype=np.float64) / HD))
    freqs = np.outer(np.arange(S, dtype=np.float64), inv)          # [S, 32]
    emb = np.concatenate([freqs, freqs], axis=1)                   # [S, 64]
    cos = np.cos(emb).astype(np.float32)
    sin = np.sin(emb).astype(np.float32)
    cos2 = np.ascontiguousarray(np.tile(cos.T, (2, 1))).astype(BF)  # [128, S]
    sinf = np.concatenate([-sin[:, :32], sin[:, 32:]], axis=1)     # sign-folded
    sin2 = np.ascontiguousarray(np.tile(sinf.T, (2, 1))).astype(BF)
    j = np.arange(128)[:, None]
    i = np.arange(SB)[None, :]
    cmask = (j <= i).astype(BF)                                    # [128, 512]
    ident = np.eye(128, dtype=BF)

    Wq4 = Wq.reshape(D, H, HD)
    Wo4 = Wo.reshape(H, HD, D)
    Wk4 = Wk.reshape(D, KV, HD)
    Wv4 = Wv.reshape(D, KV, HD)

    maps = []
    for c in range(N_CORES):
        b, g2 = c // 4, c % 4
        gh = [8 * g2 + p for p in PERM]
        wq_c = Wq4[:, gh, :].reshape(D, 512)
        wk_c = Wk4[:, [2 * g2, 2 * g2 + 1], :].reshape(D, 128)
        wv_c = Wv4[:, [2 * g2, 2 * g2 + 1], :].reshape(D, 128)
        maps.append({
            "xT": np.ascontiguousarray(x[b].T).astype(BF),
            "wqkv": np.ascontiguousarray(
                np.concatenate([wq_c, wk_c, wv_c], axis=1)).astype(BF),
            "wo": np.ascontiguousarray(Wo4[gh].reshape(512, D)).astype(BF),
            "cos2": cos2, "sin2": sin2, "cmask": cmask, "ident": ident,
        })
    return maps


_NC_CACHE = None


def kernel(x, Wq, Wk, Wv, Wo):
    global LAST_RESULT, _NC_CACHE
    x = np.asarray(x, np.float32)
    maps = host_inputs(np.asarray(x, np.float32), np.asarray(Wq, np.float32),
                       np.asarray(Wk, np.float32), np.asarray(Wv, np.float32),
                       np.asarray(Wo, np.float32))
    if _NC_CACHE is None:
        _NC_CACHE = build_nc()
    trace = bool(os.environ.get("KERNEL_TRACE"))
    try:
        res = run_bass_kernel_spmd(_NC_CACHE, maps, list(range(N_CORES)), trace=trace)
    except (ImportError, ModuleNotFoundError):
        res = run_bass_kernel_spmd(_NC_CACHE, maps, list(range(N_CORES)), trace=False)
    LAST_RESULT = res
    out = np.zeros((B, S, D), np.float32)
    for b in range(B):
        for g2 in range(4):
            out[b] += np.asarray(res.results[4 * b + g2]["y"], np.float32)
    return out



# revision 32
# speedup vs baseline: 1.0445x; 1.0445x over previous
"""GQA kernel for trn2, 8 NeuronCores — bf16, phase-interleaved.

Sharding: core c = (b, g2) with b = c//4, g2 = c%4.  Each core handles batch b
and kv heads {2*g2, 2*g2+1} (q heads 8*g2 .. 8*g2+7).  Wq/Wk/Wv column-sharded
(packed as one wqkv [D,768] tensor), Wo row-sharded; host sums the 4 partial
y outputs per batch (y is emitted bf16, upcast on host).

Everything the PE touches is bf16; psum accumulation is f32.

RoPE without any rotate-half data movement for q: score(i,j) =
rope(q)·rope(k) = [q*cos; q*sin_sw] · [k_rope; swap(k_rope)] where sin_sw is
the 32-block-swapped sign-folded sin (host-precomputed) and swap is the
32-block partition swap.  So each q head stores a 128-row tile
[q*cos; q*sin_sw] (4 DVE muls, no add, no DMA), k materializes k_rope plus
its swap via partition-offset DVE copies, and the score matmul contracts
K=128 at identical PE cost (cost model charges the moving free dim only).

Program order interleaves phases so the PE never drains: warmup matmuls on a
memset tile cover the initial weight-DMA latency and pre-ramp the PE p-state,
then A0; B0 threads into A1's chain steps; B1 into A2; B2 into A3 + C0
chunks; B3 takes C chunks as exp-latency fillers; C tail.

Per-head exps fused into one joint ACT call on an adjacent psum pair.  The
pv matmul's extra ones-row gives the softmax denominator; DVE reciprocal;
partition-broadcast via a DRAM round-trip on the idle Pool queue (bands
0..2) or a PE outer-product (band 3, low latency for the tail).  V is
PE-transposed into a bitcast bf16 psum slice.  y rows are staged in SBUF and
stored with one [128,2048] DMA per row block (HWDGE fixed cost ~640ns/DMA
makes DMA count the scarce resource, not bytes).
"""

import os
import numpy as np
import ml_dtypes

import concourse.bass as bass
import concourse.bacc as bacc
import concourse.mybir as mybir
import concourse.tile as tile
from concourse.bass_utils import run_bass_kernel_spmd

F32 = mybir.dt.float32
BF16 = mybir.dt.bfloat16

B, S, D = 2, 2048, 2048
H, KV, HD = 32, 8, 64
N_CORES = 8
SB = 512
NSB = S // SB     # 4
NDT = D // 128    # 16
NET = 4           # q e-tiles per core
PERM = [0, 4, 1, 5, 2, 6, 3, 7]
EXP = mybir.ActivationFunctionType.Exp

LAST_RESULT = None


def build_nc():
    nc = bacc.Bacc("TRN2", target_bir_lowering=False, debug=False,
                   enable_asserts=True, num_devices=N_CORES)

    xT = nc.dram_tensor("xT", [D, S], BF16, kind="ExternalInput")
    wqkv = nc.dram_tensor("wqkv", [D, 768], BF16, kind="ExternalInput")
    wo = nc.dram_tensor("wo", [512, D], BF16, kind="ExternalInput")
    csp = nc.dram_tensor("csp", [128, 2 * S], BF16, kind="ExternalInput")
    cmid = nc.dram_tensor("cmid", [128, SB + 128], BF16, kind="ExternalInput")
    y = nc.dram_tensor("y", [S, D], BF16, kind="ExternalOutput")
    rscratch = nc.dram_tensor("rscratch", [NSB, NET, 2, SB], BF16)  # internal

    with tile.TileContext(nc) as tc:
        with (
            tc.tile_pool(name="persist", bufs=1) as persist,
            tc.tile_pool(name="wpool", bufs=1) as wpool,
            tc.tile_pool(name="xpool", bufs=1) as xpool,
            tc.tile_pool(name="apool", bufs=2, space="PSUM") as apool,
            tc.tile_pool(name="scpool", bufs=2, space="PSUM") as scpool,
            tc.tile_pool(name="pvpool", bufs=1, space="PSUM") as pvpool,
            tc.tile_pool(name="atmp", bufs=3) as atmp,
            tc.tile_pool(name="epool", bufs=5) as epool,
            tc.tile_pool(name="btmp", bufs=2) as btmp,
            tc.tile_pool(name="ypool", bufs=2) as ypool,
        ):
            # ---- persistent SBUF ----
            # qT[t][sb]: [:, h, :] = [q_h*cos ; q_h*sin_sw]  (128 = 2x64 rows)
            qT_sb = [[persist.tile([128, 2, SB], BF16, name=f"qT{t}_{s_}")
                      for s_ in range(NSB)] for t in range(NET)]
            # kT{0,1}[sb]: [k_rope_h ; swap32(k_rope_h)]
            kT0_sb = [persist.tile([128, SB], BF16, name=f"kT0_{s_}")
                      for s_ in range(NSB)]
            kT1_sb = [persist.tile([128, SB], BF16, name=f"kT1_{s_}")
                      for s_ in range(NSB)]
            v_ones0 = [persist.tile([128, 4, 65], BF16, name=f"v_ones0_{s_}")
                       for s_ in range(NSB)]
            v_ones1 = [persist.tile([128, 4, 65], BF16, name=f"v_ones1_{s_}")
                       for s_ in range(NSB)]
            outT = [[persist.tile([128, SB], BF16, name=f"outT{t}_{s_}")
                     for s_ in range(NSB)] for t in range(NET)]
            csp_sb = persist.tile([128, 2 * S], BF16, name="csp_sb")
            cos_sb = csp_sb[:, 0:S]
            sinp_sb = csp_sb[:, S:2 * S]
            cmid_sb = persist.tile([128, SB + 128], BF16, name="cmid_sb")
            cmask_sb = cmid_sb[:, 0:SB]
            ident_sb = cmid_sb[:, SB:SB + 128]
            ones_col = persist.tile([128, 4, 1], BF16, name="ones_col")
            ones_sb = persist.tile([128, 64], BF16, name="ones_sb")
            # warmup source: borrow outT[0][0] (first written at B0's drain,
            # long after the warmup matmuls' last read)
            warm_sb = outT[0][0]
            nc.gpsimd.memset(ones_sb[:], 1.0)
            nc.gpsimd.memset(warm_sb[:], 0.0)
            nc.gpsimd.memset(ones_col[:], 1.0)
            for s_ in range(NSB):
                nc.vector.tensor_copy(v_ones0[s_][:, :, 64:65], ones_col[:])
                nc.vector.tensor_copy(v_ones1[s_][:, :, 64:65], ones_col[:])

            wqkv_sb = [wpool.tile([128, 768], BF16, name=f"wqkv{d}") for d in range(NDT)]
            wo_sb = [wpool.tile([128, D], BF16, name=f"wo{f}") for f in range(4)]
            xs = xpool.tile([128, NDT, S], BF16, name="xs")

            # ---- warmup: keep the PE busy + p-state ramped while the first
            # weight/x DMAs land.  Reads the memset tile, writes an unused
            # psum slot.
            warmp = apool.tile([128, SB], F32, name="warm", tag="acc")
            for _ in range(4):
                nc.tensor.matmul(warmp[0:64, 0:64], ones_sb[:], ones_sb[:],
                                 start=True, stop=True)
            for _ in range(12):
                nc.tensor.matmul(warmp[:], warm_sb[:, 0:128], warm_sb[:],
                                 start=True, stop=True)

            # ---- prefetch DMAs ----
            # HWDGE charges a fixed ~630ns per DMA instruction across ALL
            # queues, so x is fetched column-band by column-band with ONE
            # multi-block DMA per (d-range, 512-col band) instead of per-d
            # transfers: src partition blocks come from a DRAM rearrange.
            def x_band(dlo, dhi, clo, chi):
                src = xT[dlo * 128:dhi * 128, clo:chi].rearrange(
                    "(d p) c -> p d c", d=dhi - dlo, p=128)
                nc.sync.dma_start(xs[:, dlo:dhi, clo:chi], src)

            nc.scalar.dma_start(wqkv_sb[0][:], wqkv[0:128, :])
            x_band(0, 8, 0, 512)
            for d in range(1, 8):
                nc.scalar.dma_start(wqkv_sb[d][:], wqkv[d * 128:(d + 1) * 128, :])
            x_band(8, 16, 0, 512)
            for d in range(8, NDT):
                nc.scalar.dma_start(wqkv_sb[d][:], wqkv[d * 128:(d + 1) * 128, :])
            # consts: sb0 cos/sin slices first (A0's epilogues), then the rest
            nc.scalar.dma_start(csp_sb[:, 0:SB], csp[:, 0:SB])
            nc.scalar.dma_start(csp_sb[:, S:S + SB], csp[:, S:S + SB])
            nc.scalar.dma_start(cmid_sb[:], cmid[:])
            nc.scalar.dma_start(csp_sb[:, SB:S], csp[:, SB:S])
            nc.scalar.dma_start(csp_sb[:, S + SB:2 * S], csp[:, S + SB:2 * S])
            # A1's x band, then A2/A3's
            x_band(0, 8, 512, 1024)
            x_band(8, 16, 512, 1024)
            x_band(0, 16, 1024, 1536)
            x_band(0, 16, 1536, 2048)

            tri = cmask_sb[:, 0:128]

            # ---------------- phase A ----------------
            # RoPE epilogues (see module docstring):
            #  q chain t: qT[:,h,:] <- [qtmp_h*cos ; qtmp_h*sin_sw]
            #  k chain:  kt_h[0:64] = t1_h + swap32(t2'_h);
            #            kt_h[64:128] = swap32(kt_h[0:64])
            SW = ((0, 32), (32, 0), (64, 96), (96, 64))

            def rope_q(t, sb, acc, on_act):
                # generator: yields between DVE ops so a concurrent B phase
                # can slip its mask/drain DVE work into the in-order queue
                scol = slice(sb * SB, (sb + 1) * SB)
                qtmp = atmp.tile([128, SB], BF16, name="qtmp", tag="qtmp",
                                 bufs=4)
                if on_act:
                    nc.scalar.copy(qtmp[:], acc[:])
                else:
                    nc.vector.tensor_copy(qtmp[:], acc[:])
                yield
                dst = qT_sb[t][sb]
                for h in range(2):
                    hs = slice(64 * h, 64 * h + 64)
                    nc.vector.tensor_mul(dst[0:64, h, :], qtmp[hs, :],
                                         cos_sb[hs, scol])
                    nc.vector.tensor_mul(dst[64:128, h, :], qtmp[hs, :],
                                         sinp_sb[hs, scol])
                    yield

            def rope_k(sb, acc, on_act):
                scol = slice(sb * SB, (sb + 1) * SB)
                ktmp = atmp.tile([128, SB], BF16, name="qtmp", tag="qtmp",
                                 bufs=4)
                if on_act:
                    nc.scalar.copy(ktmp[:], acc[:])
                else:
                    nc.vector.tensor_copy(ktmp[:], acc[:])
                yield
                t1k = atmp.tile([128, SB], BF16, name="t1k", tag="t1k", bufs=1)
                t2k = atmp.tile([128, SB], BF16, name="t2k", tag="t2k", bufs=1)
                krot = atmp.tile([128, SB], BF16, name="krot", tag="krot", bufs=1)
                nc.vector.tensor_mul(t1k[:], ktmp[:], cos_sb[:, scol])
                nc.vector.tensor_mul(t2k[:], ktmp[:], sinp_sb[:, scol])
                yield
                for (a, b_) in SW:
                    nc.vector.tensor_copy(krot[b_:b_ + 32, :], t2k[a:a + 32, :])
                yield
                kt0, kt1 = kT0_sb[sb], kT1_sb[sb]
                nc.vector.tensor_add(kt0[0:64, :], t1k[0:64, :], krot[0:64, :])
                nc.vector.tensor_add(kt1[0:64, :], t1k[64:128, :],
                                     krot[64:128, :])
                yield
                for kt in (kt0, kt1):
                    for (a, b_) in SW[:2]:
                        nc.vector.tensor_copy(kt[64 + b_:64 + b_ + 32, :],
                                              kt[a:a + 32, :])

            def v_epilogue(sb, acc, on_act, trts):
                vtmp = atmp.tile([128, SB], BF16, name="vtmp", tag="vtmp",
                                 bufs=2)
                if on_act:
                    nc.scalar.copy(vtmp[:], acc[:])
                else:
                    nc.vector.tensor_copy(vtmp[:], acc[:])
                yield
                for u in range(4):
                    usl = slice(u * 128, (u + 1) * 128)
                    if trts is None:
                        trt = apool.tile([128, SB], F32, name="tr",
                                         tag="acc")[:]
                    else:
                        ak = trts[u]
                        trt = ak[:] if hasattr(ak, "tensor") else ak
                    tr = trt[:, 0:64].bitcast(BF16)
                    nc.tensor.transpose(tr[:], vtmp[:, usl], ident_sb[:])
                    nc.vector.tensor_copy(v_ones0[sb][:, u, 0:64], tr[:, 0:64])
                    nc.vector.tensor_copy(v_ones1[sb][:, u, 0:64], tr[:, 64:128])
                    if u % 2 == 1:
                        yield

            # chains: k first (B needs it earliest), then q0..q3, then v
            CHAINS = [("k", slice(512, 640))] + \
                     [(t, slice(t * 128, (t + 1) * 128)) for t in range(NET)] + \
                     [("v", slice(640, 768))]

            def chain_epilogue(o, sb, acc, on_act, trts=None):
                if o == "k":
                    return rope_k(sb, acc, on_act)
                elif o == "v":
                    return v_epilogue(sb, acc, on_act, trts)
                else:
                    return rope_q(o, sb, acc, on_act)

            def run_A0():
                # d-major: x tiles arrive slower than one chain consumes
                # them, so run all 6 chains per tile.  B psum slots are idle
                # here — borrow them.
                xsl = lambda d: xs[:, d, 0:512]
                scjt = scpool.tile([128, 2, SB], F32, name="scja", tag="scj")
                accs = [apool.tile([128, SB], F32, name="acc", tag="acc"),
                        apool.tile([128, SB], F32, name="acc", tag="acc"),
                        pvpool.tile([128, SB], F32, name="pva", tag="pvA"),
                        pvpool.tile([128, SB], F32, name="pvb", tag="pvB"),
                        scjt[:, 0, :], scjt[:, 1, :]]
                accof = {o: accs[ci] for ci, (o, wcol) in enumerate(CHAINS)}
                for d in range(NDT):
                    st, sp = d == 0, d == NDT - 1
                    for ci, (o, wcol) in enumerate(CHAINS):
                        ac = accof[o]
                        av = ac[:] if hasattr(ac, "tensor") else ac
                        nc.tensor.matmul(av, wqkv_sb[d][:, wcol], xsl(d),
                                         start=st, stop=sp)
                for ci, (o, wcol) in enumerate(CHAINS):
                    ac = accof[o]
                    acc = ac if not hasattr(ac, "tensor") else ac
                    if o == "v":
                        # borrow B-phase slots so A1's chains don't queue
                        # behind the transpose drains on the acc ring
                        scjt2 = scpool.tile([128, 2, SB], F32, name="scjb",
                                            tag="scj")
                        trts = [pvpool.tile([128, SB], F32, name="pvat",
                                            tag="pvA"),
                                pvpool.tile([128, SB], F32, name="pvbt",
                                            tag="pvB"),
                                scjt2[:, 0, :], scjt2[:, 1, :]]
                        for _ in chain_epilogue(o, 0, acc, True, trts):
                            pass
                    else:
                        for _ in chain_epilogue(o, 0, acc, True):
                            pass

            def gen_A(sb):
                # chain-major generator: yields after every couple of
                # matmuls so a concurrent B phase can thread these steps
                # into its exp-latency slots
                xoff = sb * SB
                xsl = lambda d: xs[:, d, xoff:xoff + 512]
                for ci, (o, wcol) in enumerate(CHAINS):
                    acc = apool.tile([128, SB], F32, name="acc", tag="acc")
                    for d0 in range(0, NDT, 2):
                        for d in (d0, d0 + 1):
                            nc.tensor.matmul(acc[:], wqkv_sb[d][:, wcol],
                                             xsl(d), start=(d == 0),
                                             stop=(d == NDT - 1))
                        yield
                    # psum->bf16 copies go to ACT for every phase: the DVE
                    # queue in the B windows is near-saturated with mask,
                    # drain, and norm work
                    yield from chain_epilogue(o, sb, acc, True)
                    yield

            # ---------------- phase C chunks ----------------
            c_state = {"ys": None}

            def emit_C_chunk(stt, db):
                srow = slice(stt * 128, (stt + 1) * 128)
                dcol = slice(db * SB, (db + 1) * SB)
                yp = apool.tile([128, SB], F32, name="yp", tag="acc")
                ssl = slice((stt % 4) * 128, (stt % 4 + 1) * 128)
                for f in range(4):
                    nc.tensor.matmul(yp[:], outT[f][stt // 4][:, ssl],
                                     wo_sb[f][:, dcol],
                                     start=(f == 0), stop=(f == 3))
                if db == 0:
                    c_state["ys"] = ypool.tile([128, D], BF16, name="ys", tag="ys")
                ys = c_state["ys"]
                if c_state.get("tail"):
                    nc.scalar.copy(ys[:, dcol], yp[:])   # ACT is idle in the tail
                else:
                    nc.vector.tensor_copy(ys[:, dcol], yp[:])
                if stt == 15:
                    # final row block: store per-db so the last DMA is small
                    # and starts as early as possible
                    nc.sync.dma_start(y[srow, dcol], ys[:, dcol])
                elif db == 3:
                    nc.sync.dma_start(y[srow, :], ys[:])

            c_chunks = []          # ready-to-emit (stt, db) list, FIFO

            # ---------------- phase B ----------------
            def emit_B(bi, fillers=(), every=4, hold_c=0):
                if bi == 0:
                    for f_ in range(4):
                        nc.sync.dma_start(wo_sb[f_][:],
                                          wo[f_ * 128:(f_ + 1) * 128, :])
                njt = 4 * bi + 4

                def sc_exp(t, jt):
                    # scores + joint exp + causal mask for one j-tile;
                    # returns what pv needs later.  Emitted one tile AHEAD
                    # of the pv consumer so the in-order PE never sits in
                    # the exp latency shadow.
                    js, ju = jt // 4, jt % 4
                    jcol = slice(ju * 128, (ju + 1) * 128)
                    ro = jt - 4 * bi
                    lo = 128 * max(ro, 0)
                    qt = qT_sb[t][bi]
                    scj = scpool.tile([128, 2, SB], F32, name="scj", tag="scj")
                    nc.tensor.matmul(scj[:, 0, lo:], kT0_sb[js][:, jcol],
                                     qt[:, 0, lo:], start=True, stop=True)
                    nc.tensor.matmul(scj[:, 1, lo:], kT1_sb[js][:, jcol],
                                     qt[:, 1, lo:], start=True, stop=True)
                    ej = epool.tile([128, 2, SB], BF16, name="ej", tag="ej")
                    nc.scalar.activation(ej[:, :, lo:], scj[:, :, lo:],
                                         EXP, scale=0.125)
                    eA, eB = ej[:, 0, :], ej[:, 1, :]
                    if ro >= 0:
                        nc.vector.tensor_mul(eA[:, lo:lo + 128],
                                             eA[:, lo:lo + 128], tri)
                        nc.vector.tensor_mul(eB[:, lo:lo + 128],
                                             eB[:, lo:lo + 128], tri)
                    return (js, ju, lo, eA, eB)

                for t in range(NET):
                    pvA = pvpool.tile([65, SB], F32, name="pvA", tag="pvA")
                    pvB = pvpool.tile([65, SB], F32, name="pvB", tag="pvB")
                    pend = sc_exp(t, 0)
                    if t > 0:
                        # cover the previous pv pair's drain latency
                        # (pvpool bufs=1) with independent filler work
                        steps = 0
                        for f in fillers:
                            while steps < 4 and f():
                                steps += 1
                            if steps >= 4:
                                break
                    for jt in range(njt):
                        nxt = sc_exp(t, jt + 1) if jt + 1 < njt else None
                        js, ju, lo, eA, eB = pend
                        st, sp = jt == 0, jt == njt - 1
                        nc.tensor.matmul(pvA[:, lo:], v_ones0[js][:, ju, :],
                                         eA[:, lo:], start=st, stop=sp)
                        nc.tensor.matmul(pvB[:, lo:], v_ones1[js][:, ju, :],
                                         eB[:, lo:], start=st, stop=sp)
                        pend = nxt
                        if jt % every == every - 1:
                            for f in fillers:
                                if f():
                                    break
                    # drains: outT halves straight from psum (partition-
                    # offset copy for the B half; Pool would queue these
                    # behind the broadcast DMAs and stall the pv ring),
                    # then denominators, then normalize
                    nc.vector.tensor_copy(outT[t][bi][0:64, :], pvA[0:64, :])
                    nc.vector.tensor_copy(outT[t][bi][64:128, :], pvB[0:64, :])
                    rAB = btmp.tile([128, 2, SB], BF16, name="rAB", tag="rAB",
                                    bufs=1)
                    with nc.allow_low_precision(reason="bf16 softmax recip"):
                        nc.vector.reciprocal(rAB[64:65, 0, :], pvA[64:65, :])
                        nc.vector.reciprocal(rAB[64:65, 1, :], pvB[64:65, :])
                    if bi == 3 and t == NET - 1:
                        # reserved chunks: independent PE work covering the
                        # final recip->bcp->norm latency chain
                        for _ in range(2):
                            if c_chunks:
                                emit_C_chunk(*c_chunks.pop(0))
                    if bi < 3 or t < NET - 1:
                        # partition-broadcast via DRAM round-trip on the idle
                        # Pool queue (frees the PE outer-product matmuls);
                        # bf16 bc also gives the norm muls DVE 2x mode.
                        # Used for every (bi,t) except the very last one:
                        # only that norm gates the tail C chunks.
                        nc.gpsimd.dma_start(rscratch[bi, t], rAB[64:65, :, :])
                        bc = btmp.tile([128, SB], BF16, name="bc", tag="bc",
                                       bufs=2)
                        nc.gpsimd.dma_start(
                            bc[0:64, :],
                            rscratch[bi, t, 0:1, :].broadcast_to((64, SB)))
                        nc.gpsimd.dma_start(
                            bc[64:128, :],
                            rscratch[bi, t, 1:2, :].broadcast_to((64, SB)))
                        nc.vector.tensor_mul(outT[t][bi][0:64, :],
                                             outT[t][bi][0:64, :], bc[0:64, :])
                        nc.vector.tensor_mul(outT[t][bi][64:128, :],
                                             outT[t][bi][64:128, :],
                                             bc[64:128, :])
                    else:
                        # the last norm feeds the tail C chunks: use the
                        # low-latency PE outer-product broadcast instead of
                        # the DMA round-trip so the tail doesn't stall.  The
                        # psum comes from the pv ring slot (its natural
                        # predecessor reads — outT copy + recip — are
                        # exactly bcp's dependencies), not the acc ring,
                        # which would chain it behind C-chunk drains.
                        bcp = apool.tile([128, SB], F32, name="bcp", tag="acc")
                        nc.tensor.matmul(bcp[0:64, :], ones_sb[64:65, :],
                                         rAB[64:65, 0, :], start=True, stop=True)
                        nc.tensor.matmul(bcp[64:128, :], ones_sb[64:65, :],
                                         rAB[64:65, 1, :], start=True, stop=True)
                        nc.vector.tensor_mul(outT[t][bi][0:64, :],
                                             outT[t][bi][0:64, :], bcp[0:64, :])
                        nc.vector.tensor_mul(outT[t][bi][64:128, :],
                                             outT[t][bi][64:128, :],
                                             bcp[64:128, :])
                # this bi's output rows are ready for phase C
                for stt in range(4 * bi, 4 * bi + 4):
                    for db in range(4):
                        c_chunks.append((stt, db))

            # ---------------- program order ----------------
            gA = {"g": None}

            def a_filler():
                if gA["g"] is None:
                    return False
                try:
                    next(gA["g"])
                    return True
                except StopIteration:
                    gA["g"] = None
                    return False

            c_floor = {"n": 0}

            def c_filler():
                if len(c_chunks) > c_floor["n"]:
                    emit_C_chunk(*c_chunks.pop(0))
                    return True
                return False

            run_A0()
            gA["g"] = gen_A(1)
            emit_B(0, fillers=[lambda: bool(a_filler()) | bool(a_filler())],
                   every=1)
            while a_filler():
                pass
            gA["g"] = gen_A(2)
            emit_B(1, fillers=[a_filler, c_filler], every=1)
            while a_filler():
                pass
            gA["g"] = gen_A(3)
            emit_B(2, fillers=[a_filler, c_filler], every=1)
            while a_filler():
                pass
            c_floor["n"] = 2   # hold 2 chunks back for the final-norm cover
            emit_B(3, fillers=[c_filler], every=3)
            c_state["tail"] = True
            while c_chunks:
                emit_C_chunk(*c_chunks.pop(0))

    nc.compile()
    return nc


def host_inputs(x, Wq, Wk, Wv, Wo):
    """Per-core input maps (8 cores)."""
    BF = ml_dtypes.bfloat16
    inv = 1.0 / (10000.0 ** (np.arange(0, HD, 2, dtype=np.float64) / HD))
    freqs = np.outer(np.arange(S, dtype=np.float64), inv)          # [S, 32]
    emb = np.concatenate([freqs, freqs], axis=1)                   # [S, 64]
    cos = np.cos(emb).astype(np.float32)
    sin = np.sin(emb).astype(np.float32)
    cos2 = np.ascontiguousarray(np.tile(cos.T, (2, 1)))            # [128, S]
    sinf = np.concatenate([-sin[:, :32], sin[:, 32:]], axis=1)     # sign-folded
    sin2 = np.ascontiguousarray(np.tile(sinf.T, (2, 1)))           # [128, S]
    # 32-block swap within each 64-row half (rows 64:128 repeat 0:64)
    sinp2 = np.concatenate([sin2[32:64], sin2[0:32],
                            sin2[96:128], sin2[64:96]], axis=0)
    csp = np.ascontiguousarray(
        np.concatenate([cos2, sinp2], axis=1)).astype(BF)          # [128, 2S]
    j = np.arange(128)[:, None]
    i = np.arange(SB)[None, :]
    cmask = (j <= i).astype(BF)                                    # [128, 512]
    ident = np.eye(128, dtype=BF)
    cmid = np.ascontiguousarray(
        np.concatenate([cmask, ident], axis=1)).astype(BF)         # [128, 640]

    Wq4 = Wq.reshape(D, H, HD)
    Wo4 = Wo.reshape(H, HD, D)
    Wk4 = Wk.reshape(D, KV, HD)
    Wv4 = Wv.reshape(D, KV, HD)

    maps = []
    for c in range(N_CORES):
        b, g2 = c // 4, c % 4
        gh = [8 * g2 + p for p in PERM]
        wq_c = Wq4[:, gh, :].reshape(D, 512)
        wk_c = Wk4[:, [2 * g2, 2 * g2 + 1], :].reshape(D, 128)
        wv_c = Wv4[:, [2 * g2, 2 * g2 + 1], :].reshape(D, 128)
        maps.append({
            "xT": np.ascontiguousarray(x[b].T).astype(BF),
            "wqkv": np.ascontiguousarray(
                np.concatenate([wq_c, wk_c, wv_c], axis=1)).astype(BF),
            "wo": np.ascontiguousarray(Wo4[gh].reshape(512, D)).astype(BF),
            "csp": csp, "cmid": cmid,
        })
    return maps


_NC_CACHE = None


def kernel(x, Wq, Wk, Wv, Wo):
    global LAST_RESULT, _NC_CACHE
    x = np.asarray(x, np.float32)
    maps = host_inputs(np.asarray(x, np.float32), np.asarray(Wq, np.float32),
                       np.asarray(Wk, np.float32), np.asarray(Wv, np.float32),
                       np.asarray(Wo, np.float32))
    if _NC_CACHE is None:
        _NC_CACHE = build_nc()
    trace = bool(os.environ.get("KERNEL_TRACE"))
    try:
        res = run_bass_kernel_spmd(_NC_CACHE, maps, list(range(N_CORES)), trace=trace)
    except (ImportError, ModuleNotFoundError):
        res = run_bass_kernel_spmd(_NC_CACHE, maps, list(range(N_CORES)), trace=False)
    LAST_RESULT = res
    out = np.zeros((B, S, D), np.float32)
    for b in range(B):
        for g2 in range(4):
            out[b] += np.asarray(res.results[4 * b + g2]["y"], np.float32)
    return out


# revision 33
# speedup vs baseline: 1.0535x; 1.0086x over previous
"""GQA kernel for trn2, 8 NeuronCores — bf16, phase-interleaved.

Sharding: core c = (b, g2) with b = c//4, g2 = c%4.  Each core handles batch b
and kv heads {2*g2, 2*g2+1} (q heads 8*g2 .. 8*g2+7).  Wq/Wk/Wv column-sharded
(packed as one wqkv [D,768] tensor), Wo row-sharded; host sums the 4 partial
y outputs per batch (y is emitted bf16, upcast on host).

Everything the PE touches is bf16; psum accumulation is f32.

RoPE without any rotate-half data movement for q: score(i,j) =
rope(q)·rope(k) = [q*cos; q*sin_sw] · [k_rope; swap(k_rope)] where sin_sw is
the 32-block-swapped sign-folded sin (host-precomputed) and swap is the
32-block partition swap.  So each q head stores a 128-row tile
[q*cos; q*sin_sw] (4 DVE muls, no add, no DMA), k materializes k_rope plus
its swap via partition-offset DVE copies, and the score matmul contracts
K=128 at identical PE cost (cost model charges the moving free dim only).

Program order interleaves phases so the PE never drains: warmup matmuls on a
memset tile cover the initial weight-DMA latency and pre-ramp the PE p-state,
then A0; B0 threads into A1's chain steps; B1 into A2; B2 into A3 + C0
chunks; B3 takes C chunks as exp-latency fillers; C tail.

Per-head exps fused into one joint ACT call on an adjacent psum pair.  The
pv matmul's extra ones-row gives the softmax denominator; DVE reciprocal;
partition-broadcast via a DRAM round-trip on the idle Pool queue (bands
0..2) or a PE outer-product (band 3, low latency for the tail).  V is
PE-transposed into a bitcast bf16 psum slice.  y rows are staged in SBUF and
stored with one [128,2048] DMA per row block (HWDGE fixed cost ~640ns/DMA
makes DMA count the scarce resource, not bytes).
"""

import os
import numpy as np
import ml_dtypes

import concourse.bass as bass
import concourse.bacc as bacc
import concourse.mybir as mybir
import concourse.tile as tile
from concourse.bass_utils import run_bass_kernel_spmd

F32 = mybir.dt.float32
BF16 = mybir.dt.bfloat16

B, S, D = 2, 2048, 2048
H, KV, HD = 32, 8, 64
N_CORES = 8
SB = 512
NSB = S // SB     # 4
NDT = D // 128    # 16
NET = 4           # q e-tiles per core
PERM = [0, 4, 1, 5, 2, 6, 3, 7]
EXP = mybir.ActivationFunctionType.Exp

LAST_RESULT = None


def build_nc():
    nc = bacc.Bacc("TRN2", target_bir_lowering=False, debug=False,
                   enable_asserts=True, num_devices=N_CORES)

    xT = nc.dram_tensor("xT", [D, S], BF16, kind="ExternalInput")
    wqkv = nc.dram_tensor("wqkv", [D, 768], BF16, kind="ExternalInput")
    wo = nc.dram_tensor("wo", [512, D], BF16, kind="ExternalInput")
    csp = nc.dram_tensor("csp", [128, 2 * S], BF16, kind="ExternalInput")
    cmid = nc.dram_tensor("cmid", [128, SB + 128], BF16, kind="ExternalInput")
    y = nc.dram_tensor("y", [S, D], BF16, kind="ExternalOutput")
    rscratch = nc.dram_tensor("rscratch", [NSB, NET, 2, SB], BF16)  # internal

    with tile.TileContext(nc) as tc:
        with (
            tc.tile_pool(name="persist", bufs=1) as persist,
            tc.tile_pool(name="wpool", bufs=1) as wpool,
            tc.tile_pool(name="xpool", bufs=1) as xpool,
            tc.tile_pool(name="apool", bufs=2, space="PSUM") as apool,
            tc.tile_pool(name="scpool", bufs=2, space="PSUM") as scpool,
            tc.tile_pool(name="pvpool", bufs=1, space="PSUM") as pvpool,
            tc.tile_pool(name="atmp", bufs=3) as atmp,
            tc.tile_pool(name="epool", bufs=5) as epool,
            tc.tile_pool(name="btmp", bufs=2) as btmp,
            tc.tile_pool(name="ypool", bufs=2) as ypool,
        ):
            # ---- persistent SBUF ----
            # qT[t][sb]: [:, h, :] = [q_h*cos ; q_h*sin_sw]  (128 = 2x64 rows)
            qT_sb = [[persist.tile([128, 2, SB], BF16, name=f"qT{t}_{s_}")
                      for s_ in range(NSB)] for t in range(NET)]
            # kT{0,1}[sb]: [k_rope_h ; swap32(k_rope_h)]
            kT0_sb = [persist.tile([128, SB], BF16, name=f"kT0_{s_}")
                      for s_ in range(NSB)]
            kT1_sb = [persist.tile([128, SB], BF16, name=f"kT1_{s_}")
                      for s_ in range(NSB)]
            v_ones0 = [persist.tile([128, 4, 65], BF16, name=f"v_ones0_{s_}")
                       for s_ in range(NSB)]
            v_ones1 = [persist.tile([128, 4, 65], BF16, name=f"v_ones1_{s_}")
                       for s_ in range(NSB)]
            outT = [[persist.tile([128, SB], BF16, name=f"outT{t}_{s_}")
                     for s_ in range(NSB)] for t in range(NET)]
            csp_sb = persist.tile([128, 2 * S], BF16, name="csp_sb")
            cos_sb = csp_sb[:, 0:S]
            sinp_sb = csp_sb[:, S:2 * S]
            cmid_sb = persist.tile([128, SB + 128], BF16, name="cmid_sb")
            cmask_sb = cmid_sb[:, 0:SB]
            ident_sb = cmid_sb[:, SB:SB + 128]
            ones_col = persist.tile([128, 4, 1], BF16, name="ones_col")
            ones_sb = persist.tile([128, 64], BF16, name="ones_sb")
            # warmup source: borrow outT[0][0] (first written at B0's drain,
            # long after the warmup matmuls' last read)
            warm_sb = outT[0][0]
            nc.gpsimd.memset(ones_sb[:], 1.0)
            nc.gpsimd.memset(warm_sb[:], 0.0)
            nc.gpsimd.memset(ones_col[:], 1.0)
            for s_ in range(NSB):
                nc.vector.tensor_copy(v_ones0[s_][:, :, 64:65], ones_col[:])
                nc.vector.tensor_copy(v_ones1[s_][:, :, 64:65], ones_col[:])

            wqkv_sb = [wpool.tile([128, 768], BF16, name=f"wqkv{d}") for d in range(NDT)]
            wo_sb = [wpool.tile([128, D], BF16, name=f"wo{f}") for f in range(4)]
            xs = xpool.tile([128, NDT, S], BF16, name="xs")

            # ---- warmup: keep the PE busy + p-state ramped while the first
            # weight/x DMAs land.  Reads the memset tile, writes an unused
            # psum slot.
            warmp = apool.tile([128, SB], F32, name="warm", tag="acc")
            for _ in range(4):
                nc.tensor.matmul(warmp[0:64, 0:64], ones_sb[:], ones_sb[:],
                                 start=True, stop=True)
            for _ in range(12):
                nc.tensor.matmul(warmp[:], warm_sb[:, 0:128], warm_sb[:],
                                 start=True, stop=True)

            # ---- prefetch DMAs ----
            # HWDGE charges a fixed ~630ns per DMA instruction across ALL
            # queues, so x is fetched column-band by column-band with ONE
            # multi-block DMA per (d-range, 512-col band) instead of per-d
            # transfers: src partition blocks come from a DRAM rearrange.
            def x_band(dlo, dhi, clo, chi):
                src = xT[dlo * 128:dhi * 128, clo:chi].rearrange(
                    "(d p) c -> p d c", d=dhi - dlo, p=128)
                nc.sync.dma_start(xs[:, dlo:dhi, clo:chi], src)

            nc.scalar.dma_start(wqkv_sb[0][:], wqkv[0:128, :])
            x_band(0, 8, 0, 512)
            for d in range(1, 8):
                nc.scalar.dma_start(wqkv_sb[d][:], wqkv[d * 128:(d + 1) * 128, :])
            x_band(8, 16, 0, 512)
            for d in range(8, NDT):
                nc.scalar.dma_start(wqkv_sb[d][:], wqkv[d * 128:(d + 1) * 128, :])
            # consts: sb0 cos/sin slices first (A0's epilogues), then the rest
            nc.scalar.dma_start(csp_sb[:, 0:SB], csp[:, 0:SB])
            nc.scalar.dma_start(csp_sb[:, S:S + SB], csp[:, S:S + SB])
            nc.scalar.dma_start(cmid_sb[:], cmid[:])
            nc.scalar.dma_start(csp_sb[:, SB:S], csp[:, SB:S])
            nc.scalar.dma_start(csp_sb[:, S + SB:2 * S], csp[:, S + SB:2 * S])
            # A1's x band, then A2/A3's
            x_band(0, 8, 512, 1024)
            x_band(8, 16, 512, 1024)
            x_band(0, 16, 1024, 1536)
            x_band(0, 16, 1536, 2048)

            tri = cmask_sb[:, 0:128]

            # ---------------- phase A ----------------
            # RoPE epilogues (see module docstring):
            #  q chain t: qT[:,h,:] <- [qtmp_h*cos ; qtmp_h*sin_sw]
            #  k chain:  kt_h[0:64] = t1_h + swap32(t2'_h);
            #            kt_h[64:128] = swap32(kt_h[0:64])
            SW = ((0, 32), (32, 0), (64, 96), (96, 64))

            def rope_q(t, sb, acc, on_act):
                # generator: yields between DVE ops so a concurrent B phase
                # can slip its mask/drain DVE work into the in-order queue
                scol = slice(sb * SB, (sb + 1) * SB)
                qtmp = atmp.tile([128, SB], BF16, name="qtmp", tag="qtmp",
                                 bufs=4)
                if on_act:
                    nc.scalar.copy(qtmp[:], acc[:])
                else:
                    nc.vector.tensor_copy(qtmp[:], acc[:])
                yield
                dst = qT_sb[t][sb]
                for h in range(2):
                    hs = slice(64 * h, 64 * h + 64)
                    nc.vector.tensor_mul(dst[0:64, h, :], qtmp[hs, :],
                                         cos_sb[hs, scol])
                    nc.vector.tensor_mul(dst[64:128, h, :], qtmp[hs, :],
                                         sinp_sb[hs, scol])
                    yield

            def rope_k(sb, acc, on_act):
                scol = slice(sb * SB, (sb + 1) * SB)
                ktmp = atmp.tile([128, SB], BF16, name="qtmp", tag="qtmp",
                                 bufs=4)
                if on_act:
                    nc.scalar.copy(ktmp[:], acc[:])
                else:
                    nc.vector.tensor_copy(ktmp[:], acc[:])
                yield
                t1k = atmp.tile([128, SB], BF16, name="t1k", tag="t1k", bufs=1)
                t2k = atmp.tile([128, SB], BF16, name="t2k", tag="t2k", bufs=1)
                krot = atmp.tile([128, SB], BF16, name="krot", tag="krot", bufs=1)
                nc.vector.tensor_mul(t1k[:], ktmp[:], cos_sb[:, scol])
                nc.vector.tensor_mul(t2k[:], ktmp[:], sinp_sb[:, scol])
                yield
                for (a, b_) in SW:
                    nc.vector.tensor_copy(krot[b_:b_ + 32, :], t2k[a:a + 32, :])
                yield
                kt0, kt1 = kT0_sb[sb], kT1_sb[sb]
                nc.vector.tensor_add(kt0[0:64, :], t1k[0:64, :], krot[0:64, :])
                nc.vector.tensor_add(kt1[0:64, :], t1k[64:128, :],
                                     krot[64:128, :])
                yield
                for kt in (kt0, kt1):
                    for (a, b_) in SW[:2]:
                        nc.vector.tensor_copy(kt[64 + b_:64 + b_ + 32, :],
                                              kt[a:a + 32, :])

            def v_epilogue(sb, acc, on_act, trts):
                vtmp = atmp.tile([128, SB], BF16, name="vtmp", tag="vtmp",
                                 bufs=2)
                if on_act:
                    nc.scalar.copy(vtmp[:], acc[:])
                else:
                    nc.vector.tensor_copy(vtmp[:], acc[:])
                yield
                for u in range(4):
                    usl = slice(u * 128, (u + 1) * 128)
                    if trts is None:
                        trt = apool.tile([128, SB], F32, name="tr",
                                         tag="acc")[:]
                    else:
                        ak = trts[u]
                        trt = ak[:] if hasattr(ak, "tensor") else ak
                    tr = trt[:, 0:64].bitcast(BF16)
                    nc.tensor.transpose(tr[:], vtmp[:, usl], ident_sb[:])
                    nc.vector.tensor_copy(v_ones0[sb][:, u, 0:64], tr[:, 0:64])
                    nc.vector.tensor_copy(v_ones1[sb][:, u, 0:64], tr[:, 64:128])
                    if u % 2 == 1:
                        yield

            # chains: k first (B needs it earliest), then q0..q3, then v
            CHAINS = [("k", slice(512, 640))] + \
                     [(t, slice(t * 128, (t + 1) * 128)) for t in range(NET)] + \
                     [("v", slice(640, 768))]

            def chain_epilogue(o, sb, acc, on_act, trts=None):
                if o == "k":
                    return rope_k(sb, acc, on_act)
                elif o == "v":
                    return v_epilogue(sb, acc, on_act, trts)
                else:
                    return rope_q(o, sb, acc, on_act)

            def run_A0():
                # d-major: x tiles arrive slower than one chain consumes
                # them, so run all 6 chains per tile.  B psum slots are idle
                # here — borrow them.
                xsl = lambda d: xs[:, d, 0:512]
                scjt = scpool.tile([128, 2, SB], F32, name="scja", tag="scj")
                accs = [apool.tile([128, SB], F32, name="acc", tag="acc"),
                        apool.tile([128, SB], F32, name="acc", tag="acc"),
                        pvpool.tile([128, SB], F32, name="pva", tag="pvA"),
                        pvpool.tile([128, SB], F32, name="pvb", tag="pvB"),
                        scjt[:, 0, :], scjt[:, 1, :]]
                accof = {o: accs[ci] for ci, (o, wcol) in enumerate(CHAINS)}
                for d in range(NDT):
                    st, sp = d == 0, d == NDT - 1
                    for ci, (o, wcol) in enumerate(CHAINS):
                        ac = accof[o]
                        av = ac[:] if hasattr(ac, "tensor") else ac
                        nc.tensor.matmul(av, wqkv_sb[d][:, wcol], xsl(d),
                                         start=st, stop=sp)
                for ci, (o, wcol) in enumerate(CHAINS):
                    ac = accof[o]
                    acc = ac if not hasattr(ac, "tensor") else ac
                    if o == "v":
                        # borrow B-phase slots so A1's chains don't queue
                        # behind the transpose drains on the acc ring
                        scjt2 = scpool.tile([128, 2, SB], F32, name="scjb",
                                            tag="scj")
                        trts = [pvpool.tile([128, SB], F32, name="pvat",
                                            tag="pvA"),
                                pvpool.tile([128, SB], F32, name="pvbt",
                                            tag="pvB"),
                                scjt2[:, 0, :], scjt2[:, 1, :]]
                        for _ in chain_epilogue(o, 0, acc, True, trts):
                            pass
                    else:
                        for _ in chain_epilogue(o, 0, acc, True):
                            pass

            def gen_A(sb):
                # chain-major generator: yields after every couple of
                # matmuls so a concurrent B phase can thread these steps
                # into its exp-latency slots
                xoff = sb * SB
                xsl = lambda d: xs[:, d, xoff:xoff + 512]
                for ci, (o, wcol) in enumerate(CHAINS):
                    acc = apool.tile([128, SB], F32, name="acc", tag="acc")
                    for d0 in range(0, NDT, 2):
                        for d in (d0, d0 + 1):
                            nc.tensor.matmul(acc[:], wqkv_sb[d][:, wcol],
                                             xsl(d), start=(d == 0),
                                             stop=(d == NDT - 1))
                        yield
                    # psum->bf16 copies go to ACT for every phase: the DVE
                    # queue in the B windows is near-saturated with mask,
                    # drain, and norm work
                    yield from chain_epilogue(o, sb, acc, True)
                    yield

            # ---------------- phase C chunks ----------------
            c_state = {"ys": None}

            def emit_C_chunk(stt, db):
                srow = slice(stt * 128, (stt + 1) * 128)
                dcol = slice(db * SB, (db + 1) * SB)
                yp = apool.tile([128, SB], F32, name="yp", tag="acc")
                ssl = slice((stt % 4) * 128, (stt % 4 + 1) * 128)
                for f in range(4):
                    nc.tensor.matmul(yp[:], outT[f][stt // 4][:, ssl],
                                     wo_sb[f][:, dcol],
                                     start=(f == 0), stop=(f == 3))
                if db == 0:
                    c_state["ys"] = ypool.tile([128, D], BF16, name="ys", tag="ys")
                ys = c_state["ys"]
                if c_state.get("tail"):
                    nc.scalar.copy(ys[:, dcol], yp[:])   # ACT is idle in the tail
                else:
                    nc.vector.tensor_copy(ys[:, dcol], yp[:])
                if stt == 15:
                    # final row block: store per-db so the last DMA is small
                    # and starts as early as possible
                    nc.sync.dma_start(y[srow, dcol], ys[:, dcol])
                elif db == 3:
                    nc.sync.dma_start(y[srow, :], ys[:])

            c_chunks = []          # ready-to-emit (stt, db) list, FIFO

            # ---------------- phase B ----------------
            def emit_B(bi, fillers=(), every=4, hold_c=0):
                if bi == 0:
                    for f_ in range(4):
                        nc.sync.dma_start(wo_sb[f_][:],
                                          wo[f_ * 128:(f_ + 1) * 128, :])
                njt = 4 * bi + 4

                def sc_exp(t, jt):
                    # scores + joint exp + causal mask for one j-tile;
                    # returns what pv needs later.  Emitted one tile AHEAD
                    # of the pv consumer so the in-order PE never sits in
                    # the exp latency shadow.
                    js, ju = jt // 4, jt % 4
                    jcol = slice(ju * 128, (ju + 1) * 128)
                    ro = jt - 4 * bi
                    lo = 128 * max(ro, 0)
                    qt = qT_sb[t][bi]
                    scj = scpool.tile([128, 2, SB], F32, name="scj", tag="scj")
                    nc.tensor.matmul(scj[:, 0, lo:], kT0_sb[js][:, jcol],
                                     qt[:, 0, lo:], start=True, stop=True)
                    nc.tensor.matmul(scj[:, 1, lo:], kT1_sb[js][:, jcol],
                                     qt[:, 1, lo:], start=True, stop=True)
                    ej = epool.tile([128, 2, SB], BF16, name="ej", tag="ej")
                    nc.scalar.activation(ej[:, :, lo:], scj[:, :, lo:],
                                         EXP, scale=0.125)
                    eA, eB = ej[:, 0, :], ej[:, 1, :]
                    if ro >= 0:
                        nc.vector.tensor_mul(eA[:, lo:lo + 128],
                                             eA[:, lo:lo + 128], tri)
                        nc.vector.tensor_mul(eB[:, lo:lo + 128],
                                             eB[:, lo:lo + 128], tri)
                    return (js, ju, lo, eA, eB)

                for t in range(NET):
                    pvA = pvpool.tile([65, SB], F32, name="pvA", tag="pvA")
                    pvB = pvpool.tile([65, SB], F32, name="pvB", tag="pvB")
                    pend = sc_exp(t, 0)
                    if t > 0:
                        # cover the previous pv pair's drain latency
                        # (pvpool bufs=1) with independent filler work
                        steps = 0
                        for f in fillers:
                            while steps < 4 and f():
                                steps += 1
                            if steps >= 4:
                                break
                    for jt in range(njt):
                        nxt = sc_exp(t, jt + 1) if jt + 1 < njt else None
                        js, ju, lo, eA, eB = pend
                        st, sp = jt == 0, jt == njt - 1
                        nc.tensor.matmul(pvA[:, lo:], v_ones0[js][:, ju, :],
                                         eA[:, lo:], start=st, stop=sp)
                        nc.tensor.matmul(pvB[:, lo:], v_ones1[js][:, ju, :],
                                         eB[:, lo:], start=st, stop=sp)
                        pend = nxt
                        if jt % every == every - 1:
                            for f in fillers:
                                if f():
                                    break
                    # drains: outT halves straight from psum (partition-
                    # offset copy for the B half; Pool would queue these
                    # behind the broadcast DMAs and stall the pv ring),
                    # then denominators, then normalize
                    nc.vector.tensor_copy(outT[t][bi][0:64, :], pvA[0:64, :])
                    nc.vector.tensor_copy(outT[t][bi][64:128, :], pvB[0:64, :])
                    rAB = btmp.tile([128, 2, SB], BF16, name="rAB", tag="rAB",
                                    bufs=1)
                    with nc.allow_low_precision(reason="bf16 softmax recip"):
                        nc.vector.reciprocal(rAB[64:65, 0, :], pvA[64:65, :])
                        nc.vector.reciprocal(rAB[64:65, 1, :], pvB[64:65, :])
                    if bi == 3 and t == NET - 1:
                        # reserved chunks: independent PE work covering the
                        # final recip->bcp->norm latency chain
                        for _ in range(2):
                            if c_chunks:
                                emit_C_chunk(*c_chunks.pop(0))
                    if bi < 3 or t < NET - 1:
                        # partition-broadcast via DRAM round-trip on the idle
                        # Pool queue (frees the PE outer-product matmuls);
                        # bf16 bc also gives the norm muls DVE 2x mode.
                        # Used for every (bi,t) except the very last one:
                        # only that norm gates the tail C chunks.
                        nc.gpsimd.dma_start(rscratch[bi, t], rAB[64:65, :, :])
                        bc = btmp.tile([128, SB], BF16, name="bc", tag="bc",
                                       bufs=2)
                        nc.gpsimd.dma_start(
                            bc[0:64, :],
                            rscratch[bi, t, 0:1, :].broadcast_to((64, SB)))
                        nc.gpsimd.dma_start(
                            bc[64:128, :],
                            rscratch[bi, t, 1:2, :].broadcast_to((64, SB)))
                        nc.vector.tensor_mul(outT[t][bi][0:64, :],
                                             outT[t][bi][0:64, :], bc[0:64, :])
                        nc.vector.tensor_mul(outT[t][bi][64:128, :],
                                             outT[t][bi][64:128, :],
                                             bc[64:128, :])
                    else:
                        # the last norm feeds the tail C chunks: use the
                        # low-latency PE outer-product broadcast instead of
                        # the DMA round-trip so the tail doesn't stall.  The
                        # psum comes from the pv ring slot (its natural
                        # predecessor reads — outT copy + recip — are
                        # exactly bcp's dependencies), not the acc ring,
                        # which would chain it behind C-chunk drains.
                        bcp = pvpool.tile([128, SB], F32, name="bcp", tag="pvA")
                        nc.tensor.matmul(bcp[0:64, :], ones_sb[64:65, :],
                                         rAB[64:65, 0, :], start=True, stop=True)
                        nc.tensor.matmul(bcp[64:128, :], ones_sb[64:65, :],
                                         rAB[64:65, 1, :], start=True, stop=True)
                        nc.vector.tensor_mul(outT[t][bi][0:64, :],
                                             outT[t][bi][0:64, :], bcp[0:64, :])
                        nc.vector.tensor_mul(outT[t][bi][64:128, :],
                                             outT[t][bi][64:128, :],
                                             bcp[64:128, :])
                # this bi's output rows are ready for phase C
                for stt in range(4 * bi, 4 * bi + 4):
                    for db in range(4):
                        c_chunks.append((stt, db))

            # ---------------- program order ----------------
            gA = {"g": None}

            def a_filler():
                if gA["g"] is None:
                    return False
                try:
                    next(gA["g"])
                    return True
                except StopIteration:
                    gA["g"] = None
                    return False

            c_floor = {"n": 0}

            def c_filler():
                if len(c_chunks) > c_floor["n"]:
                    emit_C_chunk(*c_chunks.pop(0))
                    return True
                return False

            run_A0()
            gA["g"] = gen_A(1)
            emit_B(0, fillers=[lambda: bool(a_filler()) | bool(a_filler())],
                   every=1)
            while a_filler():
                pass
            gA["g"] = gen_A(2)
            emit_B(1, fillers=[a_filler, c_filler], every=1)
            while a_filler():
                pass
            gA["g"] = gen_A(3)
            emit_B(2, fillers=[a_filler, c_filler], every=1)
            while a_filler():
                pass
            c_floor["n"] = 2   # hold 2 chunks back for the final-norm cover
            emit_B(3, fillers=[c_filler], every=3)
            c_state["tail"] = True
            while c_chunks:
                emit_C_chunk(*c_chunks.pop(0))

    nc.compile()
    return nc


def host_inputs(x, Wq, Wk, Wv, Wo):
    """Per-core input maps (8 cores)."""
    BF = ml_dtypes.bfloat16
    inv = 1.0 / (10000.0 ** (np.arange(0, HD, 2, dtype=np.float64) / HD))
    freqs = np.outer(np.arange(S, dtype=np.float64), inv)          # [S, 32]
    emb = np.concatenate([freqs, freqs], axis=1)                   # [S, 64]
    cos = np.cos(emb).astype(np.float32)
    sin = np.sin(emb).astype(np.float32)
    cos2 = np.ascontiguousarray(np.tile(cos.T, (2, 1)))            # [128, S]
    sinf = np.concatenate([-sin[:, :32], sin[:, 32:]], axis=1)     # sign-folded
    sin2 = np.ascontiguousarray(np.tile(sinf.T, (2, 1)))           # [128, S]
    # 32-block swap within each 64-row half (rows 64:128 repeat 0:64)
    sinp2 = np.concatenate([sin2[32:64], sin2[0:32],
                            sin2[96:128], sin2[64:96]], axis=0)
    csp = np.ascontiguousarray(
        np.concatenate([cos2, sinp2], axis=1)).astype(BF)          # [128, 2S]
    j = np.arange(128)[:, None]
    i = np.arange(SB)[None, :]
    cmask = (j <= i).astype(BF)                                    # [128, 512]
    ident = np.eye(128, dtype=BF)
    cmid = np.ascontiguousarray(
        np.concatenate([cmask, ident], axis=1)).astype(BF)         # [128, 640]

    Wq4 = Wq.reshape(D, H, HD)
    Wo4 = Wo.reshape(H, HD, D)
    Wk4 = Wk.reshape(D, KV, HD)
    Wv4 = Wv.reshape(D, KV, HD)

    maps = []
    for c in range(N_CORES):
        b, g2 = c // 4, c % 4
        gh = [8 * g2 + p for p in PERM]
        wq_c = Wq4[:, gh, :].reshape(D, 512)
        wk_c = Wk4[:, [2 * g2, 2 * g2 + 1], :].reshape(D, 128)
        wv_c = Wv4[:, [2 * g2, 2 * g2 + 1], :].reshape(D, 128)
        maps.append({
            "xT": np.ascontiguousarray(x[b].T).astype(BF),
            "wqkv": np.ascontiguousarray(
                np.concatenate([wq_c, wk_c, wv_c], axis=1)).astype(BF),
            "wo": np.ascontiguousarray(Wo4[gh].reshape(512, D)).astype(BF),
            "csp": csp, "cmid": cmid,
        })
    return maps


_NC_CACHE = None


def kernel(x, Wq, Wk, Wv, Wo):
    global LAST_RESULT, _NC_CACHE
    x = np.asarray(x, np.float32)
    maps = host_inputs(np.asarray(x, np.float32), np.asarray(Wq, np.float32),
                       np.asarray(Wk, np.float32), np.asarray(Wv, np.float32),
                       np.asarray(Wo, np.float32))
    if _NC_CACHE is None:
        _NC_CACHE = build_nc()
    trace = bool(os.environ.get("KERNEL_TRACE"))
    try:
        res = run_bass_kernel_spmd(_NC_CACHE, maps, list(range(N_CORES)), trace=trace)
    except (ImportError, ModuleNotFoundError):
        res = run_bass_kernel_spmd(_NC_CACHE, maps, list(range(N_CORES)), trace=False)
    LAST_RESULT = res
    out = np.zeros((B, S, D), np.float32)
    for b in range(B):
        for g2 in range(4):
            out[b] += np.asarray(res.results[4 * b + g2]["y"], np.float32)
    return out


# revision 38
# speedup vs baseline: 1.0647x; 1.0107x over previous
"""GQA kernel for trn2, 8 NeuronCores — bf16, phase-interleaved.

Sharding: core c = (b, g2) with b = c//4, g2 = c%4.  Each core handles batch b
and kv heads {2*g2, 2*g2+1} (q heads 8*g2 .. 8*g2+7).  Wq/Wk/Wv column-sharded
(packed as one wqkv [D,768] tensor), Wo row-sharded; host sums the 4 partial
y outputs per batch (y is emitted bf16, upcast on host).

Everything the PE touches is bf16; psum accumulation is f32.

RoPE without any rotate-half data movement for q: score(i,j) =
rope(q)·rope(k) = [q*cos; q*sin_sw] · [k_rope; swap(k_rope)] where sin_sw is
the 32-block-swapped sign-folded sin (host-precomputed) and swap is the
32-block partition swap.  So each q head stores a 128-row tile
[q*cos; q*sin_sw] (4 DVE muls, no add, no DMA), k materializes k_rope plus
its swap via partition-offset DVE copies, and the score matmul contracts
K=128 at identical PE cost (cost model charges the moving free dim only).

Program order interleaves phases so the PE never drains: warmup matmuls on a
memset tile cover the initial weight-DMA latency and pre-ramp the PE p-state,
then A0; B0 threads into A1's chain steps; B1 into A2; B2 into A3 + C0
chunks; B3 takes C chunks as exp-latency fillers; C tail.

Per-head exps fused into one joint ACT call on an adjacent psum pair.  The
pv matmul's extra ones-row gives the softmax denominator; DVE reciprocal;
partition-broadcast via a DRAM round-trip on the idle Pool queue (bands
0..2) or a PE outer-product (band 3, low latency for the tail).  V is
PE-transposed into a bitcast bf16 psum slice.  y rows are staged in SBUF and
stored with one [128,2048] DMA per row block (HWDGE fixed cost ~640ns/DMA
makes DMA count the scarce resource, not bytes).
"""

import os
import numpy as np
import ml_dtypes

import concourse.bass as bass
import concourse.bacc as bacc
import concourse.mybir as mybir
import concourse.tile as tile
from concourse.bass_utils import run_bass_kernel_spmd

F32 = mybir.dt.float32
BF16 = mybir.dt.bfloat16

B, S, D = 2, 2048, 2048
H, KV, HD = 32, 8, 64
N_CORES = 8
SB = 512
NSB = S // SB     # 4
NDT = D // 128    # 16
NET = 4           # q e-tiles per core
PERM = [0, 4, 1, 5, 2, 6, 3, 7]
EXP = mybir.ActivationFunctionType.Exp

LAST_RESULT = None


def build_nc():
    nc = bacc.Bacc("TRN2", target_bir_lowering=False, debug=False,
                   enable_asserts=True, num_devices=N_CORES)

    xT = nc.dram_tensor("xT", [D, S], BF16, kind="ExternalInput")
    wqkv = nc.dram_tensor("wqkv", [D, 768], BF16, kind="ExternalInput")
    wo = nc.dram_tensor("wo", [512, D], BF16, kind="ExternalInput")
    csp = nc.dram_tensor("csp", [128, 2 * S], BF16, kind="ExternalInput")
    cmid = nc.dram_tensor("cmid", [128, SB + 128 + 256], BF16, kind="ExternalInput")
    y = nc.dram_tensor("y", [S, D], BF16, kind="ExternalOutput")
    rscratch = nc.dram_tensor("rscratch", [NSB, NET, 2, SB], BF16)  # internal

    with tile.TileContext(nc) as tc:
        with (
            tc.tile_pool(name="persist", bufs=1) as persist,
            tc.tile_pool(name="wpool", bufs=1) as wpool,
            tc.tile_pool(name="xpool", bufs=1) as xpool,
            tc.tile_pool(name="apool", bufs=2, space="PSUM") as apool,
            tc.tile_pool(name="scpool", bufs=2, space="PSUM") as scpool,
            tc.tile_pool(name="pvpool", bufs=1, space="PSUM") as pvpool,
            tc.tile_pool(name="atmp", bufs=3) as atmp,
            tc.tile_pool(name="epool", bufs=5) as epool,
            tc.tile_pool(name="btmp", bufs=2) as btmp,
            tc.tile_pool(name="ypool", bufs=2) as ypool,
        ):
            # ---- persistent SBUF ----
            # qT[t][sb]: [:, h, :] = [q_h*cos ; q_h*sin_sw]  (128 = 2x64 rows)
            qT_sb = [[persist.tile([128, 2, SB], BF16, name=f"qT{t}_{s_}")
                      for s_ in range(NSB)] for t in range(NET)]
            # kT{0,1}[sb]: [k_rope_h ; swap32(k_rope_h)]
            kT0_sb = [persist.tile([128, SB], BF16, name=f"kT0_{s_}")
                      for s_ in range(NSB)]
            kT1_sb = [persist.tile([128, SB], BF16, name=f"kT1_{s_}")
                      for s_ in range(NSB)]
            v_ones0 = [persist.tile([128, 4, 65], BF16, name=f"v_ones0_{s_}")
                       for s_ in range(NSB)]
            v_ones1 = [persist.tile([128, 4, 65], BF16, name=f"v_ones1_{s_}")
                       for s_ in range(NSB)]
            outT = [[persist.tile([128, SB], BF16, name=f"outT{t}_{s_}")
                     for s_ in range(NSB)] for t in range(NET)]
            csp_sb = persist.tile([128, 2 * S], BF16, name="csp_sb")
            cos_sb = csp_sb[:, 0:S]
            sinp_sb = csp_sb[:, S:2 * S]
            cmid_sb = persist.tile([128, SB + 128 + 256], BF16, name="cmid_sb")
            cmask_sb = cmid_sb[:, 0:SB]
            ident_sb = cmid_sb[:, SB:SB + 128]
            tri2 = cmid_sb[:, SB + 128:SB + 384].rearrange(
                "p (h c) -> p h c", h=2, c=128)
            ones_col = persist.tile([128, 4, 1], BF16, name="ones_col")
            ones_sb = persist.tile([128, 64], BF16, name="ones_sb")
            # warmup source: borrow outT[0][0] (first written at B0's drain,
            # long after the warmup matmuls' last read)
            warm_sb = outT[0][0]
            nc.gpsimd.memset(ones_sb[:], 1.0)
            nc.gpsimd.memset(warm_sb[:], 0.0)
            nc.gpsimd.memset(ones_col[:], 1.0)
            for s_ in range(NSB):
                nc.vector.tensor_copy(v_ones0[s_][:, :, 64:65], ones_col[:])
                nc.vector.tensor_copy(v_ones1[s_][:, :, 64:65], ones_col[:])

            wqkv_sb = [wpool.tile([128, 768], BF16, name=f"wqkv{d}") for d in range(NDT)]
            wo_sb = [wpool.tile([128, D], BF16, name=f"wo{f}") for f in range(4)]
            xs = xpool.tile([128, NDT, S], BF16, name="xs")

            # ---- warmup: keep the PE busy + p-state ramped while the first
            # weight/x DMAs land.  Reads the memset tile, writes an unused
            # psum slot.
            warmp = apool.tile([128, SB], F32, name="warm", tag="acc")
            for _ in range(4):
                nc.tensor.matmul(warmp[0:64, 0:64], ones_sb[:], ones_sb[:],
                                 start=True, stop=True)
            for _ in range(12):
                nc.tensor.matmul(warmp[:], warm_sb[:, 0:128], warm_sb[:],
                                 start=True, stop=True)

            # ---- prefetch DMAs ----
            # HWDGE charges a fixed ~630ns per DMA instruction across ALL
            # queues, so x is fetched column-band by column-band with ONE
            # multi-block DMA per (d-range, 512-col band) instead of per-d
            # transfers: src partition blocks come from a DRAM rearrange.
            def x_band(dlo, dhi, clo, chi):
                src = xT[dlo * 128:dhi * 128, clo:chi].rearrange(
                    "(d p) c -> p d c", d=dhi - dlo, p=128)
                nc.sync.dma_start(xs[:, dlo:dhi, clo:chi], src)

            nc.scalar.dma_start(wqkv_sb[0][:], wqkv[0:128, :])
            nc.scalar.dma_start(wqkv_sb[1][:], wqkv[128:256, :])
            x_band(0, 4, 0, 512)
            for d in range(2, 6):
                nc.scalar.dma_start(wqkv_sb[d][:], wqkv[d * 128:(d + 1) * 128, :])
            x_band(4, 8, 0, 512)
            for d in range(6, 10):
                nc.scalar.dma_start(wqkv_sb[d][:], wqkv[d * 128:(d + 1) * 128, :])
            x_band(8, 12, 0, 512)
            for d in range(10, NDT):
                nc.scalar.dma_start(wqkv_sb[d][:], wqkv[d * 128:(d + 1) * 128, :])
            x_band(12, 16, 0, 512)
            # consts: sb0 cos/sin slices first (A0's epilogues), then the rest
            nc.scalar.dma_start(csp_sb[:, 0:SB], csp[:, 0:SB])
            nc.scalar.dma_start(csp_sb[:, S:S + SB], csp[:, S:S + SB])
            nc.scalar.dma_start(cmid_sb[:], cmid[:])
            nc.scalar.dma_start(csp_sb[:, SB:S], csp[:, SB:S])
            nc.scalar.dma_start(csp_sb[:, S + SB:2 * S], csp[:, S + SB:2 * S])
            # A1's x band, then A2/A3's
            x_band(0, 8, 512, 1024)
            x_band(8, 16, 512, 1024)
            x_band(0, 16, 1024, 1536)
            x_band(0, 16, 1536, 2048)

            tri = cmask_sb[:, 0:128]

            # ---------------- phase A ----------------
            # RoPE epilogues (see module docstring):
            #  q chain t: qT[:,h,:] <- [qtmp_h*cos ; qtmp_h*sin_sw]
            #  k chain:  kt_h[0:64] = t1_h + swap32(t2'_h);
            #            kt_h[64:128] = swap32(kt_h[0:64])
            SW = ((0, 32), (32, 0), (64, 96), (96, 64))

            def copy_from(acc, dst, eng):
                if eng == "act":
                    nc.scalar.copy(dst[:], acc[:])
                elif eng == "pool":
                    nc.gpsimd.tensor_copy(dst[:], acc[:])
                else:
                    nc.vector.tensor_copy(dst[:], acc[:])

            def rope_q(t, sb, acc, eng):
                # generator: yields between DVE ops so a concurrent B phase
                # can slip its mask/drain DVE work into the in-order queue
                scol = slice(sb * SB, (sb + 1) * SB)
                qtmp = atmp.tile([128, SB], BF16, name="qtmp", tag="qtmp",
                                 bufs=4)
                copy_from(acc, qtmp, eng)
                yield
                dst = qT_sb[t][sb]
                for h in range(2):
                    hs = slice(64 * h, 64 * h + 64)
                    nc.vector.tensor_mul(dst[0:64, h, :], qtmp[hs, :],
                                         cos_sb[hs, scol])
                    nc.vector.tensor_mul(dst[64:128, h, :], qtmp[hs, :],
                                         sinp_sb[hs, scol])
                    yield

            def rope_k(sb, acc, eng):
                scol = slice(sb * SB, (sb + 1) * SB)
                ktmp = atmp.tile([128, SB], BF16, name="qtmp", tag="qtmp",
                                 bufs=4)
                copy_from(acc, ktmp, eng)
                yield
                t1k = atmp.tile([128, SB], BF16, name="t1k", tag="t1k", bufs=1)
                t2k = atmp.tile([128, SB], BF16, name="t2k", tag="t2k", bufs=1)
                krot = atmp.tile([128, SB], BF16, name="krot", tag="krot", bufs=1)
                nc.vector.tensor_mul(t1k[:], ktmp[:], cos_sb[:, scol])
                nc.vector.tensor_mul(t2k[:], ktmp[:], sinp_sb[:, scol])
                yield
                for (a, b_) in SW:
                    nc.vector.tensor_copy(krot[b_:b_ + 32, :], t2k[a:a + 32, :])
                yield
                kt0, kt1 = kT0_sb[sb], kT1_sb[sb]
                nc.vector.tensor_add(kt0[0:64, :], t1k[0:64, :], krot[0:64, :])
                nc.vector.tensor_add(kt1[0:64, :], t1k[64:128, :],
                                     krot[64:128, :])
                yield
                for kt in (kt0, kt1):
                    for (a, b_) in SW[:2]:
                        nc.vector.tensor_copy(kt[64 + b_:64 + b_ + 32, :],
                                              kt[a:a + 32, :])

            def v_epilogue(sb, acc, eng, trts):
                vtmp = atmp.tile([128, SB], BF16, name="vtmp", tag="vtmp",
                                 bufs=2)
                copy_from(acc, vtmp, eng)
                yield
                for u in range(4):
                    usl = slice(u * 128, (u + 1) * 128)
                    if trts is None:
                        trt = apool.tile([128, SB], F32, name="tr",
                                         tag="acc")[:]
                    else:
                        ak = trts[u]
                        trt = ak[:] if hasattr(ak, "tensor") else ak
                    tr = trt[:, 0:64].bitcast(BF16)
                    nc.tensor.transpose(tr[:], vtmp[:, usl], ident_sb[:])
                    nc.vector.tensor_copy(v_ones0[sb][:, u, 0:64], tr[:, 0:64])
                    nc.vector.tensor_copy(v_ones1[sb][:, u, 0:64], tr[:, 64:128])
                    if u % 2 == 1:
                        yield

            # chains: k first (B needs it earliest), then q0..q3, then v
            CHAINS = [("k", slice(512, 640))] + \
                     [(t, slice(t * 128, (t + 1) * 128)) for t in range(NET)] + \
                     [("v", slice(640, 768))]

            def chain_epilogue(o, sb, acc, eng, trts=None):
                if o == "k":
                    return rope_k(sb, acc, eng)
                elif o == "v":
                    return v_epilogue(sb, acc, eng, trts)
                else:
                    return rope_q(o, sb, acc, eng)

            def run_A0():
                # d-major: x tiles arrive slower than one chain consumes
                # them, so run all 6 chains per tile.  B psum slots are idle
                # here — borrow them.
                xsl = lambda d: xs[:, d, 0:512]
                scjt = scpool.tile([128, 2, SB], F32, name="scja", tag="scj")
                accs = [apool.tile([128, SB], F32, name="acc", tag="acc"),
                        apool.tile([128, SB], F32, name="acc", tag="acc"),
                        pvpool.tile([128, SB], F32, name="pva", tag="pvA"),
                        pvpool.tile([128, SB], F32, name="pvb", tag="pvB"),
                        scjt[:, 0, :], scjt[:, 1, :]]
                accof = {o: accs[ci] for ci, (o, wcol) in enumerate(CHAINS)}
                for d in range(NDT):
                    st, sp = d == 0, d == NDT - 1
                    for ci, (o, wcol) in enumerate(CHAINS):
                        ac = accof[o]
                        av = ac[:] if hasattr(ac, "tensor") else ac
                        nc.tensor.matmul(av, wqkv_sb[d][:, wcol], xsl(d),
                                         start=st, stop=sp)
                for ci, (o, wcol) in enumerate(CHAINS):
                    ac = accof[o]
                    acc = ac if not hasattr(ac, "tensor") else ac
                    if o == "v":
                        # borrow B-phase slots so A1's chains don't queue
                        # behind the transpose drains on the acc ring
                        scjt2 = scpool.tile([128, 2, SB], F32, name="scjb",
                                            tag="scj")
                        trts = [pvpool.tile([128, SB], F32, name="pvat",
                                            tag="pvA"),
                                pvpool.tile([128, SB], F32, name="pvbt",
                                            tag="pvB"),
                                scjt2[:, 0, :], scjt2[:, 1, :]]
                        for _ in chain_epilogue(o, 0, acc, "act", trts):
                            pass
                    else:
                        for _ in chain_epilogue(o, 0, acc, "act"):
                            pass

            def gen_A(sb):
                # chain-major generator: yields after every couple of
                # matmuls so a concurrent B phase can thread these steps
                # into its exp-latency slots
                xoff = sb * SB
                xsl = lambda d: xs[:, d, xoff:xoff + 512]
                for ci, (o, wcol) in enumerate(CHAINS):
                    acc = apool.tile([128, SB], F32, name="acc", tag="acc")
                    for d0 in range(0, NDT, 2):
                        for d in (d0, d0 + 1):
                            nc.tensor.matmul(acc[:], wqkv_sb[d][:, wcol],
                                             xsl(d), start=(d == 0),
                                             stop=(d == NDT - 1))
                        yield
                    # psum->bf16 copies go to ACT (DVE is near-saturated
                    # in the B windows; Pool cannot read PSUM)
                    yield from chain_epilogue(o, sb, acc, "act")
                    yield

            # ---------------- phase C chunks ----------------
            c_state = {"ys": None}

            def emit_C_chunk(stt, db):
                srow = slice(stt * 128, (stt + 1) * 128)
                dcol = slice(db * SB, (db + 1) * SB)
                if c_state.get("tail"):
                    # after B3 the whole psum is free: rotate chunk psums
                    # across tags so the store-from-psum DMA latency never
                    # blocks the next chunk's matmuls
                    rot = c_state.get("rot", 0)
                    c_state["rot"] = rot + 1
                    kind = rot % 4
                    if kind in (0, 1):
                        yp = apool.tile([128, SB], F32, name="yp", tag="acc")[:]
                    elif kind == 2:
                        yp = pvpool.tile([128, SB], F32, name="ypA", tag="pvA")[:]
                    else:
                        yp = pvpool.tile([128, SB], F32, name="ypB", tag="pvB")[:]
                else:
                    yp = apool.tile([128, SB], F32, name="yp", tag="acc")[:]
                ssl = slice((stt % 4) * 128, (stt % 4 + 1) * 128)
                for f in range(4):
                    nc.tensor.matmul(yp, outT[f][stt // 4][:, ssl],
                                     wo_sb[f][:, dcol],
                                     start=(f == 0), stop=(f == 3))
                if db == 0:
                    c_state["ys"] = ypool.tile([128, D], BF16, name="ys", tag="ys")
                ys = c_state["ys"]
                if c_state.get("tail"):
                    nc.scalar.copy(ys[:, dcol], yp)   # ACT is idle in the tail
                else:
                    nc.vector.tensor_copy(ys[:, dcol], yp)
                if stt >= 14:
                    # final row blocks: store per-db so the last DMA is
                    # small and starts as early as possible
                    nc.sync.dma_start(y[srow, dcol], ys[:, dcol])
                elif db == 3:
                    nc.sync.dma_start(y[srow, :], ys[:])

            c_chunks = []          # ready-to-emit (stt, db) list, FIFO

            # ---------------- phase B ----------------
            def emit_B(bi, fillers=(), every=4, hold_c=0):
                if bi == 0:
                    for f_ in range(4):
                        nc.sync.dma_start(wo_sb[f_][:],
                                          wo[f_ * 128:(f_ + 1) * 128, :])
                njt = 4 * bi + 4

                def sc_exp(t, jt):
                    # scores + joint exp + causal mask for one j-tile;
                    # returns what pv needs later.  Emitted one tile AHEAD
                    # of the pv consumer so the in-order PE never sits in
                    # the exp latency shadow.
                    js, ju = jt // 4, jt % 4
                    jcol = slice(ju * 128, (ju + 1) * 128)
                    ro = jt - 4 * bi
                    lo = 128 * max(ro, 0)
                    qt = qT_sb[t][bi]
                    scj = scpool.tile([128, 2, SB], F32, name="scj", tag="scj")
                    nc.tensor.matmul(scj[:, 0, lo:], kT0_sb[js][:, jcol],
                                     qt[:, 0, lo:], start=True, stop=True)
                    nc.tensor.matmul(scj[:, 1, lo:], kT1_sb[js][:, jcol],
                                     qt[:, 1, lo:], start=True, stop=True)
                    ej = epool.tile([128, 2, SB], BF16, name="ej", tag="ej")
                    nc.scalar.activation(ej[:, :, lo:], scj[:, :, lo:],
                                         EXP, scale=0.125)
                    eA, eB = ej[:, 0, :], ej[:, 1, :]
                    if ro >= 0:
                        nc.vector.tensor_mul(ej[:, :, lo:lo + 128],
                                             ej[:, :, lo:lo + 128], tri2)
                    return (js, ju, lo, eA, eB)

                for t in range(NET):
                    pvA = pvpool.tile([65, SB], F32, name="pvA", tag="pvA")
                    pvB = pvpool.tile([65, SB], F32, name="pvB", tag="pvB")
                    pend = sc_exp(t, 0)
                    if t > 0:
                        # cover the previous pv pair's drain latency
                        # (pvpool bufs=1) with independent filler work
                        steps = 0
                        for f in fillers:
                            while steps < 4 and f():
                                steps += 1
                            if steps >= 4:
                                break
                    for jt in range(njt):
                        nxt = sc_exp(t, jt + 1) if jt + 1 < njt else None
                        js, ju, lo, eA, eB = pend
                        st, sp = jt == 0, jt == njt - 1
                        nc.tensor.matmul(pvA[:, lo:], v_ones0[js][:, ju, :],
                                         eA[:, lo:], start=st, stop=sp)
                        nc.tensor.matmul(pvB[:, lo:], v_ones1[js][:, ju, :],
                                         eB[:, lo:], start=st, stop=sp)
                        pend = nxt
                        if jt % every == every - 1:
                            for f in fillers:
                                if f():
                                    break
                    # drains: outT halves straight from psum (partition-
                    # offset copy for the B half; Pool would queue these
                    # behind the broadcast DMAs and stall the pv ring),
                    # then denominators, then normalize
                    nc.vector.tensor_copy(outT[t][bi][0:64, :], pvA[0:64, :])
                    nc.vector.tensor_copy(outT[t][bi][64:128, :], pvB[0:64, :])
                    rAB = btmp.tile([128, 2, SB], BF16, name="rAB", tag="rAB",
                                    bufs=1)
                    with nc.allow_low_precision(reason="bf16 softmax recip"):
                        nc.vector.reciprocal(rAB[64:65, 0, :], pvA[64:65, :])
                        nc.vector.reciprocal(rAB[64:65, 1, :], pvB[64:65, :])
                    if bi == 3 and t == NET - 1:
                        # reserved chunks: independent PE work covering the
                        # final recip->bcp->norm latency chain
                        for _ in range(2):
                            if c_chunks:
                                emit_C_chunk(*c_chunks.pop(0))
                    if bi < 3 or t < NET - 1:
                        # partition-broadcast via DRAM round-trip on the idle
                        # Pool queue (frees the PE outer-product matmuls);
                        # bf16 bc also gives the norm muls DVE 2x mode.
                        # Used for every (bi,t) except the very last one:
                        # only that norm gates the tail C chunks.
                        nc.gpsimd.dma_start(rscratch[bi, t], rAB[64:65, :, :])
                        bc = btmp.tile([128, SB], BF16, name="bc", tag="bc",
                                       bufs=2)
                        nc.gpsimd.dma_start(
                            bc[0:64, :],
                            rscratch[bi, t, 0:1, :].broadcast_to((64, SB)))
                        nc.gpsimd.dma_start(
                            bc[64:128, :],
                            rscratch[bi, t, 1:2, :].broadcast_to((64, SB)))
                        nc.vector.tensor_mul(outT[t][bi][:],
                                             outT[t][bi][:], bc[:])
                    else:
                        # the last norm feeds the tail C chunks: use the
                        # low-latency PE outer-product broadcast instead of
                        # the DMA round-trip so the tail doesn't stall.  The
                        # psum comes from the pv ring slot (its natural
                        # predecessor reads — outT copy + recip — are
                        # exactly bcp's dependencies), not the acc ring,
                        # which would chain it behind C-chunk drains.
                        bcp = pvpool.tile([128, SB], F32, name="bcp", tag="pvA")
                        nc.tensor.matmul(bcp[0:64, :], ones_sb[64:65, :],
                                         rAB[64:65, 0, :], start=True, stop=True)
                        nc.tensor.matmul(bcp[64:128, :], ones_sb[64:65, :],
                                         rAB[64:65, 1, :], start=True, stop=True)
                        nc.vector.tensor_mul(outT[t][bi][:],
                                             outT[t][bi][:], bcp[:])
                # this bi's output rows are ready for phase C
                for stt in range(4 * bi, 4 * bi + 4):
                    for db in range(4):
                        c_chunks.append((stt, db))

            # ---------------- program order ----------------
            gA = {"g": None}

            def a_filler():
                if gA["g"] is None:
                    return False
                try:
                    next(gA["g"])
                    return True
                except StopIteration:
                    gA["g"] = None
                    return False

            c_floor = {"n": 0}

            def c_filler():
                if len(c_chunks) > c_floor["n"]:
                    emit_C_chunk(*c_chunks.pop(0))
                    return True
                return False

            run_A0()
            gA["g"] = gen_A(1)
            emit_B(0, fillers=[lambda: bool(a_filler()) | bool(a_filler())],
                   every=1)
            while a_filler():
                pass
            gA["g"] = gen_A(2)
            emit_B(1, fillers=[a_filler, c_filler], every=1)
            while a_filler():
                pass
            gA["g"] = gen_A(3)
            emit_B(2, fillers=[a_filler, c_filler], every=1)
            while a_filler():
                pass
            c_floor["n"] = 2   # hold 2 chunks back for the final-norm cover
            emit_B(3, fillers=[c_filler], every=3)
            c_state["tail"] = True
            while c_chunks:
                emit_C_chunk(*c_chunks.pop(0))

    nc.compile()
    return nc


def host_inputs(x, Wq, Wk, Wv, Wo):
    """Per-core input maps (8 cores)."""
    BF = ml_dtypes.bfloat16
    inv = 1.0 / (10000.0 ** (np.arange(0, HD, 2, dtype=np.float64) / HD))
    freqs = np.outer(np.arange(S, dtype=np.float64), inv)          # [S, 32]
    emb = np.concatenate([freqs, freqs], axis=1)                   # [S, 64]
    cos = np.cos(emb).astype(np.float32)
    sin = np.sin(emb).astype(np.float32)
    cos2 = np.ascontiguousarray(np.tile(cos.T, (2, 1)))            # [128, S]
    sinf = np.concatenate([-sin[:, :32], sin[:, 32:]], axis=1)     # sign-folded
    sin2 = np.ascontiguousarray(np.tile(sinf.T, (2, 1)))           # [128, S]
    # 32-block swap within each 64-row half (rows 64:128 repeat 0:64)
    sinp2 = np.concatenate([sin2[32:64], sin2[0:32],
                            sin2[96:128], sin2[64:96]], axis=0)
    csp = np.ascontiguousarray(
        np.concatenate([cos2, sinp2], axis=1)).astype(BF)          # [128, 2S]
    j = np.arange(128)[:, None]
    i = np.arange(SB)[None, :]
    cmask = (j <= i).astype(BF)                                    # [128, 512]
    ident = np.eye(128, dtype=BF)
    tri = cmask[:, 0:128]
    cmid = np.ascontiguousarray(
        np.concatenate([cmask, ident, tri, tri], axis=1)).astype(BF)  # [128, 896]

    Wq4 = Wq.reshape(D, H, HD)
    Wo4 = Wo.reshape(H, HD, D)
    Wk4 = Wk.reshape(D, KV, HD)
    Wv4 = Wv.reshape(D, KV, HD)

    maps = []
    for c in range(N_CORES):
        b, g2 = c // 4, c % 4
        gh = [8 * g2 + p for p in PERM]
        wq_c = Wq4[:, gh, :].reshape(D, 512)
        wk_c = Wk4[:, [2 * g2, 2 * g2 + 1], :].reshape(D, 128)
        wv_c = Wv4[:, [2 * g2, 2 * g2 + 1], :].reshape(D, 128)
        maps.append({
            "xT": np.ascontiguousarray(x[b].T).astype(BF),
            "wqkv": np.ascontiguousarray(
                np.concatenate([wq_c, wk_c, wv_c], axis=1)).astype(BF),
            "wo": np.ascontiguousarray(Wo4[gh].reshape(512, D)).astype(BF),
            "csp": csp, "cmid": cmid,
        })
    return maps


_NC_CACHE = None


def kernel(x, Wq, Wk, Wv, Wo):
    global LAST_RESULT, _NC_CACHE
    x = np.asarray(x, np.float32)
    maps = host_inputs(np.asarray(x, np.float32), np.asarray(Wq, np.float32),
                       np.asarray(Wk, np.float32), np.asarray(Wv, np.float32),
                       np.asarray(Wo, np.float32))
    if _NC_CACHE is None:
        _NC_CACHE = build_nc()
    trace = bool(os.environ.get("KERNEL_TRACE"))
    try:
        res = run_bass_kernel_spmd(_NC_CACHE, maps, list(range(N_CORES)), trace=trace)
    except (ImportError, ModuleNotFoundError):
        res = run_bass_kernel_spmd(_NC_CACHE, maps, list(range(N_CORES)), trace=False)
    LAST_RESULT = res
    out = np.zeros((B, S, D), np.float32)
    for b in range(B):
        for g2 in range(4):
            out[b] += np.asarray(res.results[4 * b + g2]["y"], np.float32)
    return out


# revision 49
# speedup vs baseline: 1.0816x; 1.0159x over previous
"""GQA kernel for trn2, 8 NeuronCores — bf16, phase-interleaved.

Sharding: core c = (b, g2) with b = c//4, g2 = c%4.  Each core handles batch b
and kv heads {2*g2, 2*g2+1} (q heads 8*g2 .. 8*g2+7).  Wq/Wk/Wv column-sharded
(packed as one wqkv [D,768] tensor), Wo row-sharded; host sums the 4 partial
y outputs per batch (y is emitted bf16, upcast on host).

Everything the PE touches is bf16; psum accumulation is f32.

RoPE without any rotate-half data movement for q: score(i,j) =
rope(q)·rope(k) = [q*cos; q*sin_sw] · [k_rope; swap(k_rope)] where sin_sw is
the 32-block-swapped sign-folded sin (host-precomputed) and swap is the
32-block partition swap.  So each q head stores a 128-row tile
[q*cos; q*sin_sw] (4 DVE muls, no add, no DMA), k materializes k_rope plus
its swap via partition-offset DVE copies, and the score matmul contracts
K=128 at identical PE cost (cost model charges the moving free dim only).

Program order interleaves phases so the PE never drains: warmup matmuls on a
memset tile cover the initial weight-DMA latency and pre-ramp the PE p-state,
then A0; B0 threads into A1's chain steps; B1 into A2; B2 into A3 + C0
chunks; B3 takes C chunks as exp-latency fillers; C tail.

Per-head exps fused into one joint ACT call on an adjacent psum pair.  The
pv matmul's extra ones-row gives the softmax denominator; DVE reciprocal;
partition-broadcast via a DRAM round-trip on the idle Pool queue (bands
0..2) or a PE outer-product (band 3, low latency for the tail).  V is
PE-transposed into a bitcast bf16 psum slice.  y rows are staged in SBUF and
stored with one [128,2048] DMA per row block (HWDGE fixed cost ~640ns/DMA
makes DMA count the scarce resource, not bytes).
"""

import os
import numpy as np
import ml_dtypes

import concourse.bass as bass
import concourse.bacc as bacc
import concourse.mybir as mybir
import concourse.tile as tile
from concourse.bass_utils import run_bass_kernel_spmd

F32 = mybir.dt.float32
BF16 = mybir.dt.bfloat16

B, S, D = 2, 2048, 2048
H, KV, HD = 32, 8, 64
N_CORES = 8
SB = 512
NSB = S // SB     # 4
NDT = D // 128    # 16
NET = 4           # q e-tiles per core
PERM = [0, 4, 1, 5, 2, 6, 3, 7]
EXP = mybir.ActivationFunctionType.Exp

LAST_RESULT = None


def build_nc():
    nc = bacc.Bacc("TRN2", target_bir_lowering=False, debug=False,
                   enable_asserts=True, num_devices=N_CORES)

    xT = nc.dram_tensor("xT", [D, S], BF16, kind="ExternalInput")
    wqkv = nc.dram_tensor("wqkv", [D, 768], BF16, kind="ExternalInput")
    wo = nc.dram_tensor("wo", [512, D], BF16, kind="ExternalInput")
    csp = nc.dram_tensor("csp", [128, 2 * S], BF16, kind="ExternalInput")
    cmid = nc.dram_tensor("cmid", [128, SB + 128 + 256], BF16, kind="ExternalInput")
    y = nc.dram_tensor("y", [S, D], BF16, kind="ExternalOutput")
    rscratch = nc.dram_tensor("rscratch", [NSB, NET, 2, SB], BF16)  # internal

    with tile.TileContext(nc) as tc:
        with (
            tc.tile_pool(name="persist", bufs=1) as persist,
            tc.tile_pool(name="wpool", bufs=1) as wpool,
            tc.tile_pool(name="xpool", bufs=1) as xpool,
            tc.tile_pool(name="apool", bufs=2, space="PSUM") as apool,
            tc.tile_pool(name="scpool", bufs=2, space="PSUM") as scpool,
            tc.tile_pool(name="pvpool", bufs=1, space="PSUM") as pvpool,
            tc.tile_pool(name="atmp", bufs=3) as atmp,
            tc.tile_pool(name="epool", bufs=5) as epool,
            tc.tile_pool(name="btmp", bufs=2) as btmp,
            tc.tile_pool(name="ypool", bufs=2) as ypool,
        ):
            # ---- persistent SBUF ----
            # qT[t][sb]: [:, h, :] = [q_h*cos ; q_h*sin_sw]  (128 = 2x64 rows)
            qT_sb = [[persist.tile([128, 2, SB], BF16, name=f"qT{t}_{s_}")
                      for s_ in range(NSB)] for t in range(NET)]
            # kT{0,1}[sb]: [k_rope_h ; swap32(k_rope_h)]
            kT0_sb = [persist.tile([128, SB], BF16, name=f"kT0_{s_}")
                      for s_ in range(NSB)]
            kT1_sb = [persist.tile([128, SB], BF16, name=f"kT1_{s_}")
                      for s_ in range(NSB)]
            v_ones0 = [persist.tile([128, 4, 65], BF16, name=f"v_ones0_{s_}")
                       for s_ in range(NSB)]
            v_ones1 = [persist.tile([128, 4, 65], BF16, name=f"v_ones1_{s_}")
                       for s_ in range(NSB)]
            outT = [[persist.tile([128, SB], BF16, name=f"outT{t}_{s_}")
                     for s_ in range(NSB)] for t in range(NET)]
            csp_sb = persist.tile([128, 2 * S], BF16, name="csp_sb")
            cos_sb = csp_sb[:, 0:S]
            sinp_sb = csp_sb[:, S:2 * S]
            cmid_sb = persist.tile([128, SB + 128 + 256], BF16, name="cmid_sb")
            cmask_sb = cmid_sb[:, 0:SB]
            ident_sb = cmid_sb[:, SB:SB + 128]
            tri2 = cmid_sb[:, SB + 128:SB + 384].rearrange(
                "p (h c) -> p h c", h=2, c=128)
            ones_col = persist.tile([128, 4, 1], BF16, name="ones_col")
            ones_sb = persist.tile([128, 64], BF16, name="ones_sb")
            # warmup source: borrow outT[0][0] (first written at B0's drain,
            # long after the warmup matmuls' last read)
            warm_sb = outT[0][0]
            nc.gpsimd.memset(ones_sb[:], 1.0)
            nc.gpsimd.memset(warm_sb[:], 0.0)
            nc.gpsimd.memset(ones_col[:], 1.0)
            for s_ in range(NSB):
                nc.vector.tensor_copy(v_ones0[s_][:, :, 64:65], ones_col[:])
                nc.vector.tensor_copy(v_ones1[s_][:, :, 64:65], ones_col[:])

            wqkv_sb = [wpool.tile([128, 768], BF16, name=f"wqkv{d}") for d in range(NDT)]
            wo_sb = [wpool.tile([128, D], BF16, name=f"wo{f}") for f in range(4)]
            xs = xpool.tile([128, NDT, S], BF16, name="xs")

            # ---- warmup: keep the PE busy + p-state ramped while the first
            # weight/x DMAs land.  Reads the memset tile, writes an unused
            # psum slot.
            warmp = apool.tile([128, SB], F32, name="warm", tag="acc")
            for _ in range(4):
                nc.tensor.matmul(warmp[0:64, 0:64], ones_sb[:], ones_sb[:],
                                 start=True, stop=True)
            for _ in range(12):
                nc.tensor.matmul(warmp[:], warm_sb[:, 0:128], warm_sb[:],
                                 start=True, stop=True)

            # ---- prefetch DMAs ----
            # HWDGE charges a fixed ~630ns per DMA instruction across ALL
            # queues, so x is fetched column-band by column-band with ONE
            # multi-block DMA per (d-range, 512-col band) instead of per-d
            # transfers: src partition blocks come from a DRAM rearrange.
            def x_band(dlo, dhi, clo, chi):
                src = xT[dlo * 128:dhi * 128, clo:chi].rearrange(
                    "(d p) c -> p d c", d=dhi - dlo, p=128)
                nc.sync.dma_start(xs[:, dlo:dhi, clo:chi], src)

            nc.scalar.dma_start(wqkv_sb[0][:], wqkv[0:128, :])
            nc.scalar.dma_start(wqkv_sb[1][:], wqkv[128:256, :])
            x_band(0, 4, 0, 512)
            for d in range(2, 6):
                nc.scalar.dma_start(wqkv_sb[d][:], wqkv[d * 128:(d + 1) * 128, :])
            x_band(4, 8, 0, 512)
            for d in range(6, 10):
                nc.scalar.dma_start(wqkv_sb[d][:], wqkv[d * 128:(d + 1) * 128, :])
            x_band(8, 12, 0, 512)
            for d in range(10, NDT):
                nc.scalar.dma_start(wqkv_sb[d][:], wqkv[d * 128:(d + 1) * 128, :])
            x_band(12, 16, 0, 512)
            # consts: sb0 cos/sin slices first (A0's epilogues), then the rest
            nc.scalar.dma_start(csp_sb[:, 0:SB], csp[:, 0:SB])
            nc.scalar.dma_start(csp_sb[:, S:S + SB], csp[:, S:S + SB])
            nc.scalar.dma_start(cmid_sb[:], cmid[:])
            nc.scalar.dma_start(csp_sb[:, SB:S], csp[:, SB:S])
            nc.scalar.dma_start(csp_sb[:, S + SB:2 * S], csp[:, S + SB:2 * S])
            # A1's x band, then A2/A3's
            x_band(0, 8, 512, 1024)
            x_band(8, 16, 512, 1024)
            x_band(0, 16, 1024, 1536)
            x_band(0, 16, 1536, 2048)

            tri = cmask_sb[:, 0:128]

            # ---------------- phase A ----------------
            # RoPE epilogues (see module docstring):
            #  q chain t: qT[:,h,:] <- [qtmp_h*cos ; qtmp_h*sin_sw]
            #  k chain:  kt_h[0:64] = t1_h + swap32(t2'_h);
            #            kt_h[64:128] = swap32(kt_h[0:64])
            SW = ((0, 32), (32, 0), (64, 96), (96, 64))

            def copy_from(acc, dst, eng):
                if eng == "act":
                    nc.scalar.copy(dst[:], acc[:])
                elif eng == "pool":
                    nc.gpsimd.tensor_copy(dst[:], acc[:])
                else:
                    nc.vector.tensor_copy(dst[:], acc[:])

            def rope_q(t, sb, acc, eng):
                # generator: yields between DVE ops so a concurrent B phase
                # can slip its mask/drain DVE work into the in-order queue
                scol = slice(sb * SB, (sb + 1) * SB)
                qtmp = atmp.tile([128, SB], BF16, name="qtmp", tag="qtmp",
                                 bufs=4)
                copy_from(acc, qtmp, eng)
                yield
                dst = qT_sb[t][sb]
                for h in range(2):
                    hs = slice(64 * h, 64 * h + 64)
                    nc.vector.tensor_mul(dst[0:64, h, :], qtmp[hs, :],
                                         cos_sb[hs, scol])
                    nc.vector.tensor_mul(dst[64:128, h, :], qtmp[hs, :],
                                         sinp_sb[hs, scol])
                    yield

            def rope_k(sb, acc, eng):
                scol = slice(sb * SB, (sb + 1) * SB)
                ktmp = atmp.tile([128, SB], BF16, name="qtmp", tag="qtmp",
                                 bufs=4)
                copy_from(acc, ktmp, eng)
                yield
                t1k = atmp.tile([128, SB], BF16, name="t1k", tag="t1k", bufs=1)
                t2k = atmp.tile([128, SB], BF16, name="t2k", tag="t2k", bufs=1)
                krot = atmp.tile([128, SB], BF16, name="krot", tag="krot", bufs=1)
                nc.vector.tensor_mul(t1k[:], ktmp[:], cos_sb[:, scol])
                nc.vector.tensor_mul(t2k[:], ktmp[:], sinp_sb[:, scol])
                yield
                for (a, b_) in SW:
                    nc.vector.tensor_copy(krot[b_:b_ + 32, :], t2k[a:a + 32, :])
                yield
                kt0, kt1 = kT0_sb[sb], kT1_sb[sb]
                nc.vector.tensor_add(kt0[0:64, :], t1k[0:64, :], krot[0:64, :])
                nc.vector.tensor_add(kt1[0:64, :], t1k[64:128, :],
                                     krot[64:128, :])
                yield
                for kt in (kt0, kt1):
                    for (a, b_) in SW[:2]:
                        nc.vector.tensor_copy(kt[64 + b_:64 + b_ + 32, :],
                                              kt[a:a + 32, :])

            def v_epilogue(sb, acc, eng, trts):
                vtmp = atmp.tile([128, SB], BF16, name="vtmp", tag="vtmp",
                                 bufs=2)
                copy_from(acc, vtmp, eng)
                yield
                for u in range(4):
                    usl = slice(u * 128, (u + 1) * 128)
                    if trts is None:
                        trt = apool.tile([128, SB], F32, name="tr",
                                         tag="acc")[:]
                    else:
                        ak = trts[u]
                        trt = ak[:] if hasattr(ak, "tensor") else ak
                    tr = trt[:, 0:64].bitcast(BF16)
                    nc.tensor.transpose(tr[:], vtmp[:, usl], ident_sb[:])
                    nc.vector.tensor_copy(v_ones0[sb][:, u, 0:64], tr[:, 0:64])
                    nc.vector.tensor_copy(v_ones1[sb][:, u, 0:64], tr[:, 64:128])
                    if u % 2 == 1:
                        yield

            # chains: k first (B needs it earliest), then q0..q3, then v
            CHAINS = [("k", slice(512, 640))] + \
                     [(t, slice(t * 128, (t + 1) * 128)) for t in range(NET)] + \
                     [("v", slice(640, 768))]

            def chain_epilogue(o, sb, acc, eng, trts=None):
                if o == "k":
                    return rope_k(sb, acc, eng)
                elif o == "v":
                    return v_epilogue(sb, acc, eng, trts)
                else:
                    return rope_q(o, sb, acc, eng)

            def run_A0():
                # d-major: x tiles arrive slower than one chain consumes
                # them, so run all 6 chains per tile.  B psum slots are idle
                # here — borrow them.
                xsl = lambda d: xs[:, d, 0:512]
                scjt = scpool.tile([128, 2, SB], F32, name="scja", tag="scj")
                accs = [apool.tile([128, SB], F32, name="acc", tag="acc"),
                        apool.tile([128, SB], F32, name="acc", tag="acc"),
                        pvpool.tile([128, SB], F32, name="pva", tag="pvA"),
                        pvpool.tile([128, SB], F32, name="pvb", tag="pvB"),
                        scjt[:, 0, :], scjt[:, 1, :]]
                accof = {o: accs[ci] for ci, (o, wcol) in enumerate(CHAINS)}
                for d in range(NDT):
                    st, sp = d == 0, d == NDT - 1
                    for ci, (o, wcol) in enumerate(CHAINS):
                        ac = accof[o]
                        av = ac[:] if hasattr(ac, "tensor") else ac
                        nc.tensor.matmul(av, wqkv_sb[d][:, wcol], xsl(d),
                                         start=st, stop=sp)
                for ci, (o, wcol) in enumerate(CHAINS):
                    ac = accof[o]
                    acc = ac if not hasattr(ac, "tensor") else ac
                    if o == "v":
                        # borrow B-phase slots so A1's chains don't queue
                        # behind the transpose drains on the acc ring
                        scjt2 = scpool.tile([128, 2, SB], F32, name="scjb",
                                            tag="scj")
                        trts = [pvpool.tile([128, SB], F32, name="pvat",
                                            tag="pvA"),
                                pvpool.tile([128, SB], F32, name="pvbt",
                                            tag="pvB"),
                                scjt2[:, 0, :], scjt2[:, 1, :]]
                        for _ in chain_epilogue(o, 0, acc, "act", trts):
                            pass
                    else:
                        for _ in chain_epilogue(o, 0, acc, "act"):
                            pass

            def gen_A(sb):
                # chain-major generator: yields after every couple of
                # matmuls so a concurrent B phase can thread these steps
                # into its exp-latency slots
                xoff = sb * SB
                xsl = lambda d: xs[:, d, xoff:xoff + 512]
                for ci, (o, wcol) in enumerate(CHAINS):
                    acc = apool.tile([128, SB], F32, name="acc", tag="acc")
                    for d0 in range(0, NDT, 2):
                        for d in (d0, d0 + 1):
                            nc.tensor.matmul(acc[:], wqkv_sb[d][:, wcol],
                                             xsl(d), start=(d == 0),
                                             stop=(d == NDT - 1))
                        yield True   # PE work
                    # psum->bf16 copies go to ACT (DVE is near-saturated
                    # in the B windows; Pool cannot read PSUM)
                    for _ in chain_epilogue(o, sb, acc, "act"):
                        yield False  # DVE/ACT-only step
                    yield False

            # ---------------- phase C chunks ----------------
            c_state = {"ys": None}

            def emit_C_chunk(stt, db):
                srow = slice(stt * 128, (stt + 1) * 128)
                dcol = slice(db * SB, (db + 1) * SB)
                if c_state.get("tail"):
                    # after B3 the whole psum is free: rotate chunk psums
                    # across tags so the store-from-psum DMA latency never
                    # blocks the next chunk's matmuls
                    rot = c_state.get("rot", 0)
                    c_state["rot"] = rot + 1
                    kind = rot % 4
                    if kind in (0, 1):
                        yp = apool.tile([128, SB], F32, name="yp", tag="acc")[:]
                    elif kind == 2:
                        yp = pvpool.tile([128, SB], F32, name="ypA", tag="pvA")[:]
                    else:
                        yp = pvpool.tile([128, SB], F32, name="ypB", tag="pvB")[:]
                else:
                    yp = apool.tile([128, SB], F32, name="yp", tag="acc")[:]
                ssl = slice((stt % 4) * 128, (stt % 4 + 1) * 128)
                for f in range(4):
                    nc.tensor.matmul(yp, outT[f][stt // 4][:, ssl],
                                     wo_sb[f][:, dcol],
                                     start=(f == 0), stop=(f == 3))
                if db == 0:
                    c_state["ys"] = ypool.tile([128, D], BF16, name="ys", tag="ys")
                ys = c_state["ys"]
                if c_state.get("tail"):
                    nc.scalar.copy(ys[:, dcol], yp)   # ACT is idle in the tail
                else:
                    nc.vector.tensor_copy(ys[:, dcol], yp)
                if stt >= 14:
                    # final row blocks: store per-db so the last DMA is
                    # small and starts as early as possible
                    nc.sync.dma_start(y[srow, dcol], ys[:, dcol])
                elif db == 3:
                    nc.sync.dma_start(y[srow, :], ys[:])

            c_chunks = []          # ready-to-emit (stt, db) list, FIFO

            # ---------------- phase B ----------------
            def emit_B(bi, fillers=(), every=4, hold_c=0):
                if bi == 0:
                    for f_ in range(4):
                        nc.sync.dma_start(wo_sb[f_][:],
                                          wo[f_ * 128:(f_ + 1) * 128, :])
                njt = 4 * bi + 4

                def sc_exp(t, jt):
                    # scores + joint exp + causal mask for one j-tile;
                    # returns what pv needs later.  Emitted one tile AHEAD
                    # of the pv consumer so the in-order PE never sits in
                    # the exp latency shadow.
                    js, ju = jt // 4, jt % 4
                    jcol = slice(ju * 128, (ju + 1) * 128)
                    ro = jt - 4 * bi
                    lo = 128 * max(ro, 0)
                    qt = qT_sb[t][bi]
                    scj = scpool.tile([128, 2, SB], F32, name="scj", tag="scj")
                    nc.tensor.matmul(scj[:, 0, lo:], kT0_sb[js][:, jcol],
                                     qt[:, 0, lo:], start=True, stop=True)
                    nc.tensor.matmul(scj[:, 1, lo:], kT1_sb[js][:, jcol],
                                     qt[:, 1, lo:], start=True, stop=True)
                    ej = epool.tile([128, 2, SB], BF16, name="ej", tag="ej")
                    nc.scalar.activation(ej[:, :, lo:], scj[:, :, lo:],
                                         EXP, scale=0.125)
                    eA, eB = ej[:, 0, :], ej[:, 1, :]
                    if ro >= 0:
                        nc.vector.tensor_mul(ej[:, :, lo:lo + 128],
                                             ej[:, :, lo:lo + 128], tri2)
                    return (js, ju, lo, eA, eB)

                for t in range(NET):
                    pvA = pvpool.tile([65, SB], F32, name="pvA", tag="pvA")
                    pvB = pvpool.tile([65, SB], F32, name="pvB", tag="pvB")
                    pend = sc_exp(t, 0)
                    if t > 0:
                        # cover the previous pv pair's drain latency
                        # (pvpool bufs=1) with filler work that actually
                        # feeds the PE: epilogue-only generator steps don't
                        # count (and C chunks count double)
                        steps = 0
                        pulls = 0
                        for f in fillers:
                            while steps < 4 and pulls < 12:
                                r = f()
                                pulls += 1
                                if not r:
                                    break
                                if r == "mm":
                                    steps += 1
                                elif r is True:   # c_filler chunk
                                    steps += 2
                            if steps >= 4:
                                break
                    for jt in range(njt):
                        nxt = sc_exp(t, jt + 1) if jt + 1 < njt else None
                        js, ju, lo, eA, eB = pend
                        st, sp = jt == 0, jt == njt - 1
                        nc.tensor.matmul(pvA[:, lo:], v_ones0[js][:, ju, :],
                                         eA[:, lo:], start=st, stop=sp)
                        nc.tensor.matmul(pvB[:, lo:], v_ones1[js][:, ju, :],
                                         eB[:, lo:], start=st, stop=sp)
                        pend = nxt
                        if jt % every == every - 1:
                            for f in fillers:
                                if f():
                                    break
                    # drains: the pv psum pair is handed back only after
                    # outT copies + denominator reciprocals; split the four
                    # ops across DVE (A half) and ACT (B half) so the ring
                    # turnaround halves
                    rAB = btmp.tile([128, 2, SB], BF16, name="rAB", tag="rAB",
                                    bufs=1)
                    nc.vector.tensor_copy(outT[t][bi][0:64, :], pvA[0:64, :])
                    nc.scalar.copy(outT[t][bi][64:128, :], pvB[0:64, :])
                    with nc.allow_low_precision(reason="bf16 softmax recip"):
                        nc.vector.reciprocal(rAB[64:65, 0, :], pvA[64:65, :])
                        nc.vector.reciprocal(rAB[64:65, 1, :], pvB[64:65, :])
                    if bi == 3 and t == NET - 1:
                        # reserved chunks: independent PE work covering the
                        # final recip->bcp->norm latency chain
                        for _ in range(2):
                            if c_chunks:
                                emit_C_chunk(*c_chunks.pop(0))
                    if bi < 3 or t < NET - 1:
                        # partition-broadcast via DRAM round-trip on the idle
                        # Pool queue (frees the PE outer-product matmuls);
                        # bf16 bc also gives the norm muls DVE 2x mode.
                        # Used for every (bi,t) except the very last one:
                        # only that norm gates the tail C chunks.
                        nc.gpsimd.dma_start(rscratch[bi, t], rAB[64:65, :, :])
                        bc = btmp.tile([128, SB], BF16, name="bc", tag="bc",
                                       bufs=2)
                        nc.gpsimd.dma_start(
                            bc[0:64, :],
                            rscratch[bi, t, 0:1, :].broadcast_to((64, SB)))
                        nc.gpsimd.dma_start(
                            bc[64:128, :],
                            rscratch[bi, t, 1:2, :].broadcast_to((64, SB)))
                        nc.vector.tensor_mul(outT[t][bi][:],
                                             outT[t][bi][:], bc[:])
                    else:
                        # the last norm feeds the tail C chunks: use the
                        # low-latency PE outer-product broadcast instead of
                        # the DMA round-trip so the tail doesn't stall.  The
                        # psum comes from the pv ring slot (its natural
                        # predecessor reads — outT copy + recip — are
                        # exactly bcp's dependencies), not the acc ring,
                        # which would chain it behind C-chunk drains.
                        bcp = pvpool.tile([128, SB], F32, name="bcp", tag="pvA")
                        nc.tensor.matmul(bcp[0:64, :], ones_sb[64:65, :],
                                         rAB[64:65, 0, :], start=True, stop=True)
                        nc.tensor.matmul(bcp[64:128, :], ones_sb[64:65, :],
                                         rAB[64:65, 1, :], start=True, stop=True)
                        nc.vector.tensor_mul(outT[t][bi][:],
                                             outT[t][bi][:], bcp[:])
                # this bi's output rows are ready for phase C
                for stt in range(4 * bi, 4 * bi + 4):
                    for db in range(4):
                        c_chunks.append((stt, db))

            # ---------------- program order ----------------
            gA = {"g": None}

            def a_filler():
                # returns "mm" for a PE-matmul step, "ep" for an
                # epilogue-only step, False when exhausted
                if gA["g"] is None:
                    return False
                try:
                    tag = next(gA["g"])
                    return "mm" if tag else "ep"
                except StopIteration:
                    gA["g"] = None
                    return False

            c_floor = {"n": 0}

            def c_filler():
                if len(c_chunks) > c_floor["n"]:
                    emit_C_chunk(*c_chunks.pop(0))
                    return True
                return False

            run_A0()
            gA["g"] = gen_A(1)
            emit_B(0, fillers=[lambda: bool(a_filler()) | bool(a_filler())],
                   every=1)
            while a_filler():
                pass
            gA["g"] = gen_A(2)
            emit_B(1, fillers=[a_filler, c_filler], every=1)
            while a_filler():
                pass
            gA["g"] = gen_A(3)
            emit_B(2, fillers=[a_filler, c_filler], every=1)
            while a_filler():
                pass
            c_floor["n"] = 2   # hold 2 chunks back for the final-norm cover
            emit_B(3, fillers=[c_filler], every=3)
            c_state["tail"] = True
            while c_chunks:
                emit_C_chunk(*c_chunks.pop(0))

    nc.compile()
    return nc


def host_inputs(x, Wq, Wk, Wv, Wo):
    """Per-core input maps (8 cores)."""
    BF = ml_dtypes.bfloat16
    inv = 1.0 / (10000.0 ** (np.arange(0, HD, 2, dtype=np.float64) / HD))
    freqs = np.outer(np.arange(S, dtype=np.float64), inv)          # [S, 32]
    emb = np.concatenate([freqs, freqs], axis=1)                   # [S, 64]
    cos = np.cos(emb).astype(np.float32)
    sin = np.sin(emb).astype(np.float32)
    cos2 = np.ascontiguousarray(np.tile(cos.T, (2, 1)))            # [128, S]
    sinf = np.concatenate([-sin[:, :32], sin[:, 32:]], axis=1)     # sign-folded
    sin2 = np.ascontiguousarray(np.tile(sinf.T, (2, 1)))           # [128, S]
    # 32-block swap within each 64-row half (rows 64:128 repeat 0:64)
    sinp2 = np.concatenate([sin2[32:64], sin2[0:32],
                            sin2[96:128], sin2[64:96]], axis=0)
    csp = np.ascontiguousarray(
        np.concatenate([cos2, sinp2], axis=1)).astype(BF)          # [128, 2S]
    j = np.arange(128)[:, None]
    i = np.arange(SB)[None, :]
    cmask = (j <= i).astype(BF)                                    # [128, 512]
    ident = np.eye(128, dtype=BF)
    tri = cmask[:, 0:128]
    cmid = np.ascontiguousarray(
        np.concatenate([cmask, ident, tri, tri], axis=1)).astype(BF)  # [128, 896]

    Wq4 = Wq.reshape(D, H, HD)
    Wo4 = Wo.reshape(H, HD, D)
    Wk4 = Wk.reshape(D, KV, HD)
    Wv4 = Wv.reshape(D, KV, HD)

    maps = []
    for c in range(N_CORES):
        b, g2 = c // 4, c % 4
        gh = [8 * g2 + p for p in PERM]
        wq_c = Wq4[:, gh, :].reshape(D, 512)
        wk_c = Wk4[:, [2 * g2, 2 * g2 + 1], :].reshape(D, 128)
        wv_c = Wv4[:, [2 * g2, 2 * g2 + 1], :].reshape(D, 128)
        maps.append({
            "xT": np.ascontiguousarray(x[b].T).astype(BF),
            "wqkv": np.ascontiguousarray(
                np.concatenate([wq_c, wk_c, wv_c], axis=1)).astype(BF),
            "wo": np.ascontiguousarray(Wo4[gh].reshape(512, D)).astype(BF),
            "csp": csp, "cmid": cmid,
        })
    return maps


_NC_CACHE = None


def kernel(x, Wq, Wk, Wv, Wo):
    global LAST_RESULT, _NC_CACHE
    x = np.asarray(x, np.float32)
    maps = host_inputs(np.asarray(x, np.float32), np.asarray(Wq, np.float32),
                       np.asarray(Wk, np.float32), np.asarray(Wv, np.float32),
                       np.asarray(Wo, np.float32))
    if _NC_CACHE is None:
        _NC_CACHE = build_nc()
    trace = bool(os.environ.get("KERNEL_TRACE"))
    try:
        res = run_bass_kernel_spmd(_NC_CACHE, maps, list(range(N_CORES)), trace=trace)
    except (ImportError, ModuleNotFoundError):
        res = run_bass_kernel_spmd(_NC_CACHE, maps, list(range(N_CORES)), trace=False)
    LAST_RESULT = res
    out = np.zeros((B, S, D), np.float32)
    for b in range(B):
        for g2 in range(4):
            out[b] += np.asarray(res.results[4 * b + g2]["y"], np.float32)
    return out


# revision 60
# speedup vs baseline: 1.1078x; 1.0242x over previous
"""GQA kernel for trn2, 8 NeuronCores — bf16, phase-interleaved.

Sharding: core c = (b, g2) with b = c//4, g2 = c%4.  Each core handles batch b
and kv heads {2*g2, 2*g2+1} (q heads 8*g2 .. 8*g2+7).  Wq/Wk/Wv column-sharded
(packed as one wqkv [D,768] tensor), Wo row-sharded; host sums the 4 partial
y outputs per batch (y is emitted bf16, upcast on host).

Everything the PE touches is bf16; psum accumulation is f32.

RoPE without any rotate-half data movement for q: score(i,j) =
rope(q)·rope(k) = [q*cos; q*sin_sw] · [k_rope; swap(k_rope)] where sin_sw is
the 32-block-swapped sign-folded sin (host-precomputed) and swap is the
32-block partition swap.  So each q head stores a 128-row tile
[q*cos; q*sin_sw] (4 DVE muls, no add, no DMA), k materializes k_rope plus
its swap via partition-offset DVE copies, and the score matmul contracts
K=128 at identical PE cost (cost model charges the moving free dim only).

Program order interleaves phases so the PE never drains: warmup matmuls on a
memset tile cover the initial weight-DMA latency and pre-ramp the PE p-state,
then A0; B0 threads into A1's chain steps; B1 into A2; B2 into A3 + C0
chunks; B3 takes C chunks as exp-latency fillers; C tail.

Per-head exps fused into one joint ACT call on an adjacent psum pair.  The
pv matmul's extra ones-row gives the softmax denominator; DVE reciprocal;
partition-broadcast via a DRAM round-trip on the idle Pool queue (bands
0..2) or a PE outer-product (band 3, low latency for the tail).  V is
PE-transposed into a bitcast bf16 psum slice.  y rows are staged in SBUF and
stored with one [128,2048] DMA per row block (HWDGE fixed cost ~640ns/DMA
makes DMA count the scarce resource, not bytes).
"""

import os
import numpy as np
import ml_dtypes

import concourse.bass as bass
import concourse.bacc as bacc
import concourse.mybir as mybir
import concourse.tile as tile
from concourse.bass_utils import run_bass_kernel_spmd

F32 = mybir.dt.float32
BF16 = mybir.dt.bfloat16

B, S, D = 2, 2048, 2048
H, KV, HD = 32, 8, 64
N_CORES = 8
SB = 512
NSB = S // SB     # 4
NDT = D // 128    # 16
NET = 4           # q e-tiles per core
PERM = [0, 4, 1, 5, 2, 6, 3, 7]
EXP = mybir.ActivationFunctionType.Exp

LAST_RESULT = None


def build_nc():
    nc = bacc.Bacc("TRN2", target_bir_lowering=False, debug=False,
                   enable_asserts=True, num_devices=N_CORES)

    xT = nc.dram_tensor("xT", [D, S], BF16, kind="ExternalInput")
    wqkv = nc.dram_tensor("wqkv", [D, 768], BF16, kind="ExternalInput")
    wo = nc.dram_tensor("wo", [512, D], BF16, kind="ExternalInput")
    csp = nc.dram_tensor("csp", [128, 2 * S], BF16, kind="ExternalInput")
    cmid = nc.dram_tensor("cmid", [128, SB + 128 + 256], BF16, kind="ExternalInput")
    y = nc.dram_tensor("y", [S, D], BF16, kind="ExternalOutput")
    rscratch = nc.dram_tensor("rscratch", [NSB, NET, 2, SB], BF16)  # internal

    with tile.TileContext(nc) as tc:
        with (
            tc.tile_pool(name="persist", bufs=1) as persist,
            tc.tile_pool(name="wpool", bufs=1) as wpool,
            tc.tile_pool(name="xpool", bufs=1) as xpool,
            tc.tile_pool(name="apool", bufs=2, space="PSUM") as apool,
            tc.tile_pool(name="scpool", bufs=2, space="PSUM") as scpool,
            tc.tile_pool(name="pvpool", bufs=1, space="PSUM") as pvpool,
            tc.tile_pool(name="atmp", bufs=3) as atmp,
            tc.tile_pool(name="epool", bufs=5) as epool,
            tc.tile_pool(name="btmp", bufs=2) as btmp,
            tc.tile_pool(name="ypool", bufs=2) as ypool,
        ):
            # ---- persistent SBUF ----
            # qT[t][sb]: [:, h, :] = [q_h*cos ; q_h*sin_sw]  (128 = 2x64 rows)
            qT_sb = [[persist.tile([128, 2, SB], BF16, name=f"qT{t}_{s_}")
                      for s_ in range(NSB)] for t in range(NET)]
            # kT{0,1}[sb]: [k_rope_h ; swap32(k_rope_h)]
            kT0_sb = [persist.tile([128, SB], BF16, name=f"kT0_{s_}")
                      for s_ in range(NSB)]
            kT1_sb = [persist.tile([128, SB], BF16, name=f"kT1_{s_}")
                      for s_ in range(NSB)]
            v_ones0 = [persist.tile([128, 4, 65], BF16, name=f"v_ones0_{s_}")
                       for s_ in range(NSB)]
            v_ones1 = [persist.tile([128, 4, 65], BF16, name=f"v_ones1_{s_}")
                       for s_ in range(NSB)]
            outT = [[persist.tile([128, SB], BF16, name=f"outT{t}_{s_}")
                     for s_ in range(NSB)] for t in range(NET)]
            csp_sb = persist.tile([128, 2 * S], BF16, name="csp_sb")
            cos_sb = csp_sb[:, 0:S]
            sinp_sb = csp_sb[:, S:2 * S]
            cmid_sb = persist.tile([128, SB + 128 + 256], BF16, name="cmid_sb")
            cmask_sb = cmid_sb[:, 0:SB]
            ident_sb = cmid_sb[:, SB:SB + 128]
            tri2 = cmid_sb[:, SB + 128:SB + 384].rearrange(
                "p (h c) -> p h c", h=2, c=128)
            ones_col = persist.tile([128, 4, 1], BF16, name="ones_col")
            ones_sb = persist.tile([128, 64], BF16, name="ones_sb")
            # warmup source: borrow outT[0][0] (first written at B0's drain,
            # long after the warmup matmuls' last read)
            warm_sb = outT[0][0]
            nc.gpsimd.memset(ones_sb[:], 1.0)
            nc.gpsimd.memset(warm_sb[:], 0.0)
            nc.gpsimd.memset(ones_col[:], 1.0)
            for s_ in range(NSB):
                nc.vector.tensor_copy(v_ones0[s_][:, :, 64:65], ones_col[:])
                nc.vector.tensor_copy(v_ones1[s_][:, :, 64:65], ones_col[:])

            wqkv_sb = [wpool.tile([128, 768], BF16, name=f"wqkv{d}") for d in range(NDT)]
            wo_sb = [wpool.tile([128, D], BF16, name=f"wo{f}") for f in range(4)]
            xs = xpool.tile([128, NDT, S], BF16, name="xs")

            # ---- warmup: keep the PE busy + p-state ramped while the first
            # weight/x DMAs land.  Reads the memset tile, writes an unused
            # psum slot.
            warmp = apool.tile([128, SB], F32, name="warm", tag="acc")
            for _ in range(4):
                nc.tensor.matmul(warmp[0:64, 0:64], ones_sb[:], ones_sb[:],
                                 start=True, stop=True)
            for _ in range(12):
                nc.tensor.matmul(warmp[:], warm_sb[:, 0:128], warm_sb[:],
                                 start=True, stop=True)

            # ---- prefetch DMAs ----
            # HWDGE charges a fixed ~630ns per DMA instruction across ALL
            # queues, so x is fetched column-band by column-band with ONE
            # multi-block DMA per (d-range, 512-col band) instead of per-d
            # transfers: src partition blocks come from a DRAM rearrange.
            def x_band(dlo, dhi, clo, chi):
                src = xT[dlo * 128:dhi * 128, clo:chi].rearrange(
                    "(d p) c -> p d c", d=dhi - dlo, p=128)
                nc.sync.dma_start(xs[:, dlo:dhi, clo:chi], src)

            nc.scalar.dma_start(wqkv_sb[0][:], wqkv[0:128, :])
            nc.scalar.dma_start(wqkv_sb[1][:], wqkv[128:256, :])
            x_band(0, 4, 0, 512)
            for d in range(2, 6):
                nc.scalar.dma_start(wqkv_sb[d][:], wqkv[d * 128:(d + 1) * 128, :])
            x_band(4, 8, 0, 512)
            for d in range(6, 10):
                nc.scalar.dma_start(wqkv_sb[d][:], wqkv[d * 128:(d + 1) * 128, :])
            x_band(8, 12, 0, 512)
            for d in range(10, NDT):
                nc.scalar.dma_start(wqkv_sb[d][:], wqkv[d * 128:(d + 1) * 128, :])
            x_band(12, 16, 0, 512)
            # consts: sb0 cos/sin slices first (A0's epilogues), then the rest
            nc.scalar.dma_start(csp_sb[:, 0:SB], csp[:, 0:SB])
            nc.scalar.dma_start(csp_sb[:, S:S + SB], csp[:, S:S + SB])
            nc.scalar.dma_start(cmid_sb[:], cmid[:])
            nc.scalar.dma_start(csp_sb[:, SB:S], csp[:, SB:S])
            nc.scalar.dma_start(csp_sb[:, S + SB:2 * S], csp[:, S + SB:2 * S])
            # A1's x band, then A2/A3's
            x_band(0, 8, 512, 1024)
            x_band(8, 16, 512, 1024)
            x_band(0, 16, 1024, 1536)
            x_band(0, 16, 1536, 2048)

            tri = cmask_sb[:, 0:128]

            # ---------------- phase A ----------------
            # RoPE epilogues (see module docstring):
            #  q chain t: qT[:,h,:] <- [qtmp_h*cos ; qtmp_h*sin_sw]
            #  k chain:  kt_h[0:64] = t1_h + swap32(t2'_h);
            #            kt_h[64:128] = swap32(kt_h[0:64])
            SW = ((0, 32), (32, 0), (64, 96), (96, 64))

            def copy_from(acc, dst, eng):
                if eng == "act":
                    nc.scalar.copy(dst[:], acc[:])
                elif eng == "pool":
                    nc.gpsimd.tensor_copy(dst[:], acc[:])
                else:
                    nc.vector.tensor_copy(dst[:], acc[:])

            def rope_q(t, sb, acc, eng):
                # generator: yields between DVE ops so a concurrent B phase
                # can slip its mask/drain DVE work into the in-order queue
                scol = slice(sb * SB, (sb + 1) * SB)
                qtmp = atmp.tile([128, SB], BF16, name="qtmp", tag="qtmp",
                                 bufs=4)
                copy_from(acc, qtmp, eng)
                yield
                dst = qT_sb[t][sb]
                for h in range(2):
                    hs = slice(64 * h, 64 * h + 64)
                    nc.vector.tensor_mul(dst[0:64, h, :], qtmp[hs, :],
                                         cos_sb[hs, scol])
                    nc.vector.tensor_mul(dst[64:128, h, :], qtmp[hs, :],
                                         sinp_sb[hs, scol])
                    yield

            def rope_k(sb, acc, eng):
                scol = slice(sb * SB, (sb + 1) * SB)
                ktmp = atmp.tile([128, SB], BF16, name="qtmp", tag="qtmp",
                                 bufs=4)
                copy_from(acc, ktmp, eng)
                yield
                t1k = atmp.tile([128, SB], BF16, name="t1k", tag="t1k", bufs=1)
                t2k = atmp.tile([128, SB], BF16, name="t2k", tag="t2k", bufs=1)
                krot = atmp.tile([128, SB], BF16, name="krot", tag="krot", bufs=1)
                nc.vector.tensor_mul(t1k[:], ktmp[:], cos_sb[:, scol])
                nc.vector.tensor_mul(t2k[:], ktmp[:], sinp_sb[:, scol])
                yield
                for (a, b_) in SW:
                    nc.vector.tensor_copy(krot[b_:b_ + 32, :], t2k[a:a + 32, :])
                yield
                kt0, kt1 = kT0_sb[sb], kT1_sb[sb]
                nc.vector.tensor_add(kt0[0:64, :], t1k[0:64, :], krot[0:64, :])
                nc.vector.tensor_add(kt1[0:64, :], t1k[64:128, :],
                                     krot[64:128, :])
                yield
                for kt in (kt0, kt1):
                    for (a, b_) in SW[:2]:
                        nc.vector.tensor_copy(kt[64 + b_:64 + b_ + 32, :],
                                              kt[a:a + 32, :])

            def v_epilogue(sb, acc, eng, trts):
                vtmp = atmp.tile([128, SB], BF16, name="vtmp", tag="vtmp",
                                 bufs=2)
                copy_from(acc, vtmp, eng)
                yield
                for u in range(4):
                    usl = slice(u * 128, (u + 1) * 128)
                    if trts is None:
                        trt = apool.tile([128, SB], F32, name="tr",
                                         tag="acc")[:]
                    else:
                        ak = trts[u]
                        trt = ak[:] if hasattr(ak, "tensor") else ak
                    tr = trt[:, 0:64].bitcast(BF16)
                    nc.tensor.transpose(tr[:], vtmp[:, usl], ident_sb[:])
                    nc.vector.tensor_copy(v_ones0[sb][:, u, 0:64], tr[:, 0:64])
                    nc.vector.tensor_copy(v_ones1[sb][:, u, 0:64], tr[:, 64:128])
                    if u % 2 == 1:
                        yield

            # chains: k first (B needs it earliest), then q0..q3, then v
            CHAINS = [("k", slice(512, 640))] + \
                     [(t, slice(t * 128, (t + 1) * 128)) for t in range(NET)] + \
                     [("v", slice(640, 768))]

            def chain_epilogue(o, sb, acc, eng, trts=None):
                if o == "k":
                    return rope_k(sb, acc, eng)
                elif o == "v":
                    return v_epilogue(sb, acc, eng, trts)
                else:
                    return rope_q(o, sb, acc, eng)

            def run_A0():
                # d-major: x tiles arrive slower than one chain consumes
                # them, so run all 6 chains per tile.  B psum slots are idle
                # here — borrow them.
                xsl = lambda d: xs[:, d, 0:512]
                scjt = scpool.tile([128, 2, SB], F32, name="scja", tag="scj")
                accs = [apool.tile([128, SB], F32, name="acc", tag="acc"),
                        apool.tile([128, SB], F32, name="acc", tag="acc"),
                        pvpool.tile([128, SB], F32, name="pva", tag="pvA"),
                        pvpool.tile([128, SB], F32, name="pvb", tag="pvB"),
                        scjt[:, 0, :], scjt[:, 1, :]]
                accof = {o: accs[ci] for ci, (o, wcol) in enumerate(CHAINS)}
                for d in range(NDT):
                    st, sp = d == 0, d == NDT - 1
                    for ci, (o, wcol) in enumerate(CHAINS):
                        ac = accof[o]
                        av = ac[:] if hasattr(ac, "tensor") else ac
                        nc.tensor.matmul(av, wqkv_sb[d][:, wcol], xsl(d),
                                         start=st, stop=sp)
                for ci, (o, wcol) in enumerate(CHAINS):
                    ac = accof[o]
                    acc = ac if not hasattr(ac, "tensor") else ac
                    if o == "v":
                        # borrow B-phase slots so A1's chains don't queue
                        # behind the transpose drains on the acc ring
                        scjt2 = scpool.tile([128, 2, SB], F32, name="scjb",
                                            tag="scj")
                        trts = [pvpool.tile([128, SB], F32, name="pvat",
                                            tag="pvA"),
                                pvpool.tile([128, SB], F32, name="pvbt",
                                            tag="pvB"),
                                scjt2[:, 0, :], scjt2[:, 1, :]]
                        for _ in chain_epilogue(o, 0, acc, "act", trts):
                            pass
                    else:
                        for _ in chain_epilogue(o, 0, acc, "act"):
                            pass

            def gen_A(sb):
                # chain-major generator: yields after every couple of
                # matmuls so a concurrent B phase can thread these steps
                # into its exp-latency slots
                xoff = sb * SB
                xsl = lambda d: xs[:, d, xoff:xoff + 512]
                for ci, (o, wcol) in enumerate(CHAINS):
                    acc = apool.tile([128, SB], F32, name="acc", tag="acc")
                    for d0 in range(0, NDT, 2):
                        for d in (d0, d0 + 1):
                            nc.tensor.matmul(acc[:], wqkv_sb[d][:, wcol],
                                             xsl(d), start=(d == 0),
                                             stop=(d == NDT - 1))
                        yield True   # PE work
                    # psum->bf16 copies go to ACT (DVE is near-saturated
                    # in the B windows; Pool cannot read PSUM)
                    for _ in chain_epilogue(o, sb, acc, "act"):
                        yield False  # DVE/ACT-only step
                    yield False

            # ---------------- phase C chunks ----------------
            c_state = {"ys": None}

            def emit_C_chunk(stt, db):
                srow = slice(stt * 128, (stt + 1) * 128)
                dcol = slice(db * SB, (db + 1) * SB)
                if c_state.get("tail"):
                    # after B3 the whole psum is free: rotate chunk psums
                    # across tags so the store-from-psum DMA latency never
                    # blocks the next chunk's matmuls
                    rot = c_state.get("rot", 0)
                    c_state["rot"] = rot + 1
                    kind = rot % 4
                    if kind in (0, 1):
                        yp = apool.tile([128, SB], F32, name="yp", tag="acc")[:]
                    elif kind == 2:
                        yp = pvpool.tile([128, SB], F32, name="ypA", tag="pvA")[:]
                    else:
                        yp = pvpool.tile([128, SB], F32, name="ypB", tag="pvB")[:]
                else:
                    yp = apool.tile([128, SB], F32, name="yp", tag="acc")[:]
                ssl = slice((stt % 4) * 128, (stt % 4 + 1) * 128)
                for f in range(4):
                    nc.tensor.matmul(yp, outT[f][stt // 4][:, ssl],
                                     wo_sb[f][:, dcol],
                                     start=(f == 0), stop=(f == 3))
                if db == 0:
                    c_state["ys"] = ypool.tile([128, D], BF16, name="ys", tag="ys")
                ys = c_state["ys"]
                if c_state.get("tail"):
                    nc.scalar.copy(ys[:, dcol], yp)   # ACT is idle in the tail
                else:
                    nc.vector.tensor_copy(ys[:, dcol], yp)
                if stt >= 14:
                    # final row blocks: store per-db so the last DMA is
                    # small and starts as early as possible
                    nc.sync.dma_start(y[srow, dcol], ys[:, dcol])
                elif db == 3:
                    nc.sync.dma_start(y[srow, :], ys[:])

            c_chunks = []          # ready-to-emit (stt, db) list, FIFO

            # ---------------- phase B ----------------
            def emit_B(bi, fillers=(), every=4, hold_c=0):
                if bi == 0:
                    for f_ in range(4):
                        nc.sync.dma_start(wo_sb[f_][:],
                                          wo[f_ * 128:(f_ + 1) * 128, :])
                njt = 4 * bi + 4

                def sc_exp(t, jt):
                    # scores + joint exp + causal mask for one j-tile;
                    # returns what pv needs later.  Emitted one tile AHEAD
                    # of the pv consumer so the in-order PE never sits in
                    # the exp latency shadow.
                    js, ju = jt // 4, jt % 4
                    jcol = slice(ju * 128, (ju + 1) * 128)
                    ro = jt - 4 * bi
                    lo = 128 * max(ro, 0)
                    qt = qT_sb[t][bi]
                    scj = scpool.tile([128, 2, SB], F32, name="scj", tag="scj")
                    import contextlib
                    hp = tc.high_priority(offset=100)
                    with hp:
                        nc.tensor.matmul(scj[:, 0, lo:], kT0_sb[js][:, jcol],
                                         qt[:, 0, lo:], start=True, stop=True)
                        nc.tensor.matmul(scj[:, 1, lo:], kT1_sb[js][:, jcol],
                                         qt[:, 1, lo:], start=True, stop=True)
                        ej = epool.tile([128, 2, SB], BF16, name="ej", tag="ej")
                        nc.scalar.activation(ej[:, :, lo:], scj[:, :, lo:],
                                             EXP, scale=0.125)
                    eA, eB = ej[:, 0, :], ej[:, 1, :]
                    if ro >= 0:
                        # the mask mul gates pv: sort it (and the whole
                        # score/exp stream) ahead of drains and fillers
                        with tc.high_priority(offset=100):
                            nc.vector.tensor_mul(ej[:, :, lo:lo + 128],
                                                 ej[:, :, lo:lo + 128], tri2)
                    return (js, ju, lo, eA, eB)

                for t in range(NET):
                    pvA = pvpool.tile([65, SB], F32, name="pvA", tag="pvA")
                    pvB = pvpool.tile([65, SB], F32, name="pvB", tag="pvB")
                    # depth-2 score pipeline: two tiles of genuine PE work
                    # overlap the previous pair's drain, and the new tile's
                    # exp latency is fully hidden
                    pend = sc_exp(t, 0)
                    pend1 = sc_exp(t, 1)
                    if t > 0:
                        # cover the previous pv pair's drain latency
                        # (pvpool bufs=1) with filler work that actually
                        # feeds the PE: epilogue-only generator steps don't
                        # count (and C chunks count double)
                        steps = 0
                        pulls = 0
                        for f in fillers:
                            while steps < 4 and pulls < 12:
                                r = f()
                                pulls += 1
                                if not r:
                                    break
                                if r == "mm":
                                    steps += 1
                                elif r is True:   # c_filler chunk
                                    steps += 2
                            if steps >= 4:
                                break
                    for jt in range(njt):
                        js, ju, lo, eA, eB = pend
                        st, sp = jt == 0, jt == njt - 1
                        nc.tensor.matmul(pvA[:, lo:], v_ones0[js][:, ju, :],
                                         eA[:, lo:], start=st, stop=sp)
                        nc.tensor.matmul(pvB[:, lo:], v_ones1[js][:, ju, :],
                                         eB[:, lo:], start=st, stop=sp)
                        pend = pend1
                        pend1 = sc_exp(t, jt + 2) if jt + 2 < njt else None
                        if jt % every == every - 1:
                            for f in fillers:
                                if f():
                                    break
                    # drains: the pv psum pair is handed back only after
                    # outT copies + denominator reciprocals; split the four
                    # ops across DVE (A half) and ACT (B half) so the ring
                    # turnaround halves
                    rAB = btmp.tile([128, 2, SB], BF16, name="rAB", tag="rAB",
                                    bufs=1)
                    nc.vector.tensor_copy(outT[t][bi][0:64, :], pvA[0:64, :])
                    nc.scalar.copy(outT[t][bi][64:128, :], pvB[0:64, :])
                    with nc.allow_low_precision(reason="bf16 softmax recip"):
                        nc.vector.reciprocal(rAB[64:65, 0, :], pvA[64:65, :])
                        nc.vector.reciprocal(rAB[64:65, 1, :], pvB[64:65, :])
                    if bi == 3 and t == NET - 1:
                        # reserved chunks: independent PE work covering the
                        # final recip->bcp->norm latency chain
                        for _ in range(2):
                            if c_chunks:
                                emit_C_chunk(*c_chunks.pop(0))
                    if bi < 3 or t < NET - 1:
                        # partition-broadcast via DRAM round-trip on the idle
                        # Pool queue (frees the PE outer-product matmuls);
                        # bf16 bc also gives the norm muls DVE 2x mode.
                        # Used for every (bi,t) except the very last one:
                        # only that norm gates the tail C chunks.
                        nc.gpsimd.dma_start(rscratch[bi, t], rAB[64:65, :, :])
                        bc = btmp.tile([128, SB], BF16, name="bc", tag="bc",
                                       bufs=2)
                        nc.gpsimd.dma_start(
                            bc[0:64, :],
                            rscratch[bi, t, 0:1, :].broadcast_to((64, SB)))
                        nc.gpsimd.dma_start(
                            bc[64:128, :],
                            rscratch[bi, t, 1:2, :].broadcast_to((64, SB)))
                        nc.vector.tensor_mul(outT[t][bi][:],
                                             outT[t][bi][:], bc[:])
                    else:
                        # the last norm feeds the tail C chunks: use the
                        # low-latency PE outer-product broadcast instead of
                        # the DMA round-trip so the tail doesn't stall.  The
                        # psum comes from the pv ring slot (its natural
                        # predecessor reads — outT copy + recip — are
                        # exactly bcp's dependencies), not the acc ring,
                        # which would chain it behind C-chunk drains.
                        bcp = pvpool.tile([128, SB], F32, name="bcp", tag="pvA")
                        nc.tensor.matmul(bcp[0:64, :], ones_sb[64:65, :],
                                         rAB[64:65, 0, :], start=True, stop=True)
                        nc.tensor.matmul(bcp[64:128, :], ones_sb[64:65, :],
                                         rAB[64:65, 1, :], start=True, stop=True)
                        nc.vector.tensor_mul(outT[t][bi][:],
                                             outT[t][bi][:], bcp[:])
                # this bi's output rows are ready for phase C
                for stt in range(4 * bi, 4 * bi + 4):
                    for db in range(4):
                        c_chunks.append((stt, db))

            # ---------------- program order ----------------
            gA = {"g": None}

            def a_filler():
                # returns "mm" for a PE-matmul step, "ep" for an
                # epilogue-only step, False when exhausted
                if gA["g"] is None:
                    return False
                try:
                    tag = next(gA["g"])
                    return "mm" if tag else "ep"
                except StopIteration:
                    gA["g"] = None
                    return False

            c_floor = {"n": 0}

            def c_filler():
                if len(c_chunks) > c_floor["n"]:
                    emit_C_chunk(*c_chunks.pop(0))
                    return True
                return False

            run_A0()
            gA["g"] = gen_A(1)
            emit_B(0, fillers=[lambda: bool(a_filler()) | bool(a_filler())],
                   every=1)
            while a_filler():
                pass
            gA["g"] = gen_A(2)
            emit_B(1, fillers=[a_filler, c_filler], every=1)
            while a_filler():
                pass
            gA["g"] = gen_A(3)
            emit_B(2, fillers=[a_filler, c_filler], every=1)
            while a_filler():
                pass
            c_floor["n"] = 2   # hold 2 chunks back for the final-norm cover
            emit_B(3, fillers=[c_filler], every=3)
            c_state["tail"] = True
            while c_chunks:
                emit_C_chunk(*c_chunks.pop(0))

    nc.compile()
    return nc


def host_inputs(x, Wq, Wk, Wv, Wo):
    """Per-core input maps (8 cores)."""
    BF = ml_dtypes.bfloat16
    inv = 1.0 / (10000.0 ** (np.arange(0, HD, 2, dtype=np.float64) / HD))
    freqs = np.outer(np.arange(S, dtype=np.float64), inv)          # [S, 32]
    emb = np.concatenate([freqs, freqs], axis=1)                   # [S, 64]
    cos = np.cos(emb).astype(np.float32)
    sin = np.sin(emb).astype(np.float32)
    cos2 = np.ascontiguousarray(np.tile(cos.T, (2, 1)))            # [128, S]
    sinf = np.concatenate([-sin[:, :32], sin[:, 32:]], axis=1)     # sign-folded
    sin2 = np.ascontiguousarray(np.tile(sinf.T, (2, 1)))           # [128, S]
    # 32-block swap within each 64-row half (rows 64:128 repeat 0:64)
    sinp2 = np.concatenate([sin2[32:64], sin2[0:32],
                            sin2[96:128], sin2[64:96]], axis=0)
    csp = np.ascontiguousarray(
        np.concatenate([cos2, sinp2], axis=1)).astype(BF)          # [128, 2S]
    j = np.arange(128)[:, None]
    i = np.arange(SB)[None, :]
    cmask = (j <= i).astype(BF)                                    # [128, 512]
    ident = np.eye(128, dtype=BF)
    tri = cmask[:, 0:128]
    cmid = np.ascontiguousarray(
        np.concatenate([cmask, ident, tri, tri], axis=1)).astype(BF)  # [128, 896]

    Wq4 = Wq.reshape(D, H, HD)
    Wo4 = Wo.reshape(H, HD, D)
    Wk4 = Wk.reshape(D, KV, HD)
    Wv4 = Wv.reshape(D, KV, HD)

    maps = []
    for c in range(N_CORES):
        b, g2 = c // 4, c % 4
        gh = [8 * g2 + p for p in PERM]
        wq_c = Wq4[:, gh, :].reshape(D, 512)
        wk_c = Wk4[:, [2 * g2, 2 * g2 + 1], :].reshape(D, 128)
        wv_c = Wv4[:, [2 * g2, 2 * g2 + 1], :].reshape(D, 128)
        maps.append({
            "xT": np.ascontiguousarray(x[b].T).astype(BF),
            "wqkv": np.ascontiguousarray(
                np.concatenate([wq_c, wk_c, wv_c], axis=1)).astype(BF),
            "wo": np.ascontiguousarray(Wo4[gh].reshape(512, D)).astype(BF),
            "csp": csp, "cmid": cmid,
        })
    return maps


_NC_CACHE = None


def kernel(x, Wq, Wk, Wv, Wo):
    global LAST_RESULT, _NC_CACHE
    x = np.asarray(x, np.float32)
    maps = host_inputs(np.asarray(x, np.float32), np.asarray(Wq, np.float32),
                       np.asarray(Wk, np.float32), np.asarray(Wv, np.float32),
                       np.asarray(Wo, np.float32))
    if _NC_CACHE is None:
        _NC_CACHE = build_nc()
    trace = bool(os.environ.get("KERNEL_TRACE"))
    try:
        res = run_bass_kernel_spmd(_NC_CACHE, maps, list(range(N_CORES)), trace=trace)
    except (ImportError, ModuleNotFoundError):
        res = run_bass_kernel_spmd(_NC_CACHE, maps, list(range(N_CORES)), trace=False)
    LAST_RESULT = res
    out = np.zeros((B, S, D), np.float32)
    for b in range(B):
        for g2 in range(4):
            out[b] += np.asarray(res.results[4 * b + g2]["y"], np.float32)
    return out
